# revision 4
# baseline (speedup 1.0000x reference)
"""Trainium2 Bass kernel for nn_KANLayer (piecewise-constant KAN forward).

Math: reference computes out[t,i] = sum_j f[i,j,m(x_tj)] where m = segment(x) in
0..8 and f[i,j,m] = c_m + c_{m+1} + c_{m+2} (9-valued selection -> exact rank
8 + constant; K = 8*512 = 4096 is the minimal bf16 contraction).

This kernel instead runs the selection in fp8-e4m3 with DoubleRow perf mode
(2 fp8 weights/cell, K=256 per matmul at 0.5 cyc/col -> ~2x bf16 rate):
    out[t,i] = base_i + (1/a) * [ sum_m R8[i,j,m]*onehot_m(t,j)          (18 DR units)
                                 + (B1hi+B1lo)[i,j]*(m_tj-4)             (4 DR units)
                                 + (B2hi+B2lo)[i,j]*(m_tj-4)^2 ]         (4 DR units)
where R8 = e4m3(a*(f - affine fit over {1, m-4, (m-4)^2})) and the affine
coefficients are carried in two e4m3 planes each (hi + lo = ~9-bit mantissa).
All selection operands ((m-4), (m-4)^2, one-hots) are fp8-EXACT values, so the
only noise is table quantization: measured 1.3e-2 rel err end-to-end on the
reference seed (threshold 2e-2). Host does the fit/quantization and ships raw
e4m3 bytes; the device never rounds.

Orientation is flipped vs the bf16 version: the fp8 table pairs are the
STATIONARY operand, reused across a whole 2048-token PSUM residency (4 banks),
so the 256-col DoubleRow LDWEIGHTS (213ns) hides under 4 N=512 matmuls
(~4x120ns). Per core: 4 out-block passes x 26 units x 4 token groups = 416
DR matmuls. Planes are [128j, jc, tok] fp8 in SBUF: 6 one-hot planes shipped
by DMA, 3 built by DVE/GpSimd is_equal, (m-4)^2 squared on ACT.

Sharding: data-parallel over tokens, 2048 per core; tables replicated.
Output is written [out_block, 128i, tok] f32 (big contiguous DMA runs) and
transposed to [tok, i] on host.
"""

from contextlib import ExitStack

import numpy as np
import ml_dtypes

import concourse.bass as bass  # noqa: F401
import concourse.tile as tile
from concourse import bacc, mybir
from concourse.bass_utils import run_bass_kernel_spmd

N_CORES = 8
TOK = 2048          # tokens per core
IN_F = 512
OUT_F = 512
JC = IN_F // 128    # 4 j-chunks of 128
NPASS = OUT_F // 128  # 4 out-block passes
NTG = 4             # token groups per psum residency
TGW = TOK // NTG    # 512 tokens per matmul (moving free dim, DR max)
NU = 26             # DR units per pass: 4 lin + 18 onehot + 4 quad
FP8 = mybir.dt.float8e4
BF16 = mybir.dt.bfloat16
F32 = mybir.dt.float32
E4NP = mybir.dt.np(FP8)  # ml_dtypes.float8_e4m3 (TRN: bias 7, max 240)

SHIP = [0, 1, 2, 3, 4, 5]   # one-hot planes shipped via DMA
BUILD_DVE = [6, 8]          # built on vector engine
BUILD_GP = [7]              # built on gpsimd

# unit -> (plane kind, jc-pair q). planes: 'seg' (m-4), 'oh<m>', 'qp' ((m-4)^2)
# order: lin first (plane ready at t0), one-hot m ascending, quad last
# (gives ACT time to square). lin/quad have hi+lo table passes on the same
# plane; one-hot m covers units 4+2m, 5+2m.
_UNITS = []
for part in range(2):                      # lin hi, lin lo
    for q in range(2):
        _UNITS.append(("seg", q))
for m in range(9):
    for q in range(2):
        _UNITS.append((f"oh{m}", q))
for part in range(2):                      # quad hi, quad lo
    for q in range(2):
        _UNITS.append(("qp", q))
assert len(_UNITS) == NU

_PROGRAM_CACHE = {}


def _build_program():
    nc = bacc.Bacc("TRN2", target_bir_lowering=False, debug=False)

    segc_d = nc.dram_tensor("segc", [128, JC, TOK], FP8, kind="ExternalInput").ap()
    ohp_d = nc.dram_tensor(
        "ohp", [128, len(SHIP), JC, TOK], FP8, kind="ExternalInput"
    ).ap()
    g_d = nc.dram_tensor("g", [128, NU, 2, NPASS, 128], FP8, kind="ExternalInput").ap()
    sb_d = nc.dram_tensor("sb", [128, 1 + NPASS], F32, kind="ExternalInput").ap()
    out_d = nc.dram_tensor("out", [NPASS, 128, TOK], F32, kind="ExternalOutput").ap()

    with tile.TileContext(nc) as tc, ExitStack() as ctx:
        wm_pool = ctx.enter_context(tc.tile_pool(name="wm", bufs=1))
        seg_pool = ctx.enter_context(tc.tile_pool(name="seg", bufs=1))
        plane_pool = ctx.enter_context(tc.tile_pool(name="plane", bufs=1))
        g_pool = ctx.enter_context(tc.tile_pool(name="g", bufs=1))
        sb_pool = ctx.enter_context(tc.tile_pool(name="sb", bufs=1))
        out_pool = ctx.enter_context(tc.tile_pool(name="out", bufs=2))
        psum_pool = ctx.enter_context(tc.tile_pool(name="psum", bufs=2, space="PSUM"))

        # --- PE warmup on a zeroed scratch tile: keeps the HAM activity
        # window busy from t=0 so the clock is 2.4 GHz when real MMs start.
        # Warmup targets pass-0's psum tile — its start=True group re-clears.
        wm = wm_pool.tile([128, 384], BF16, name="wm")
        nc.vector.memset(wm[:], 0.0)

        # --- input DMAs. sync ring: segc pieces then shipped one-hot planes
        # (consumption order); scalar ring: g table in unit-order pieces.
        segc_t = seg_pool.tile([128, JC, TOK], FP8, name="segc")
        for jc in range(JC):
            nc.sync.dma_start(segc_t[:, jc, :], segc_d[:, jc, :])

        oh_ts = [
            plane_pool.tile([128, JC, TOK], FP8, name=f"oh{m}") for m in range(9)
        ]
        for s, m in enumerate(SHIP):
            nc.sync.dma_start(oh_ts[m][:], ohp_d[:, s])

        g_t = g_pool.tile([128, NU, 2, NPASS, 128], FP8, name="g")
        g_cuts = [0, 4, 10, 16, 22, NU]
        for a, b in zip(g_cuts[:-1], g_cuts[1:]):
            nc.scalar.dma_start(g_t[:, a:b], g_d[:, a:b])

        sb_t = sb_pool.tile([128, 1 + NPASS], F32, name="sb")
        nc.gpsimd.dma_start(sb_t[:], sb_d[:])

        # --- plane builds: (m-4)^2 on ACT; remaining one-hots on DVE/GpSimd.
        qp_t = plane_pool.tile([128, JC, TOK], FP8, name="qp")
        for jc in range(JC):
            nc.scalar.square(qp_t[:, jc, :], segc_t[:, jc, :])
        for m in BUILD_DVE:
            for jc in range(JC):
                nc.vector.tensor_scalar(
                    oh_ts[m][:, jc, :], segc_t[:, jc, :],
                    float(m - 4), None, mybir.AluOpType.is_equal,
                )
        for m in BUILD_GP:
            for jc in range(JC):
                nc.gpsimd.tensor_scalar(
                    oh_ts[m][:, jc, :], segc_t[:, jc, :],
                    float(m - 4), None, mybir.AluOpType.is_equal,
                )

        planes = {"seg": segc_t, "qp": qp_t}
        for m in range(9):
            planes[f"oh{m}"] = oh_ts[m]

        # --- main: per out-block pass, accumulate all 26 DR units into a
        # 4-bank [128, 2048] psum; stationary table pair is reused across the
        # 4 token-group matmuls so LDWEIGHTS hides under the MM stream.
        for ob in range(NPASS):
            ps = psum_pool.tile([128, TOK], F32, name="ps")
            if ob == 0:
                for _ in range(20):
                    nc.tensor.matmul(
                        ps[:, :256], wm[:, :128], wm[:, 128:384],
                        start=True, stop=True, skip_group_check=True,
                    )
            for u, (pk, q) in enumerate(_UNITS):
                pl = planes[pk]
                lhsT = g_t[:, u, :, ob, :]
                for tg in range(NTG):
                    nc.tensor.matmul(
                        ps[:, tg * TGW:(tg + 1) * TGW],
                        lhsT,
                        pl[:, 2 * q:2 * q + 2, tg * TGW:(tg + 1) * TGW],
                        start=(u == 0),
                        stop=(u == NU - 1),
                        perf_mode=mybir.MatmulPerfMode.DoubleRow,
                    )
            ot = out_pool.tile([128, TOK], F32, name="ot")
            nc.vector.tensor_scalar(
                ot[:], ps[:], sb_t[:, 0:1], sb_t[:, 1 + ob:2 + ob],
                mybir.AluOpType.mult, mybir.AluOpType.add,
            )
            eng = nc.sync if ob % 2 == 0 else nc.scalar
            eng.dma_start(out_d[ob], ot[:])

    nc.compile()
    return nc


def _get_program():
    if "nc" not in _PROGRAM_CACHE:
        _PROGRAM_CACHE["nc"] = _build_program()
    return _PROGRAM_CACHE["nc"]


def _q8(v, a):
    """e4m3 bytes of a*v (clipped to TRN max 240)."""
    return np.clip(v * a, -240.0, 240.0).astype(E4NP)


def _plane_dev(arr):
    """[T_all, IN] -> [128, JC, T_all] device layout (j = jc*128 + p)."""
    return np.ascontiguousarray(arr.T.reshape(JC, 128, -1).transpose(1, 0, 2))


def _pack_pair(tab_b):
    """e4m3 [OUT, IN] -> [128p, 2q, 2e, NPASS, 128col] stationary layout."""
    t = tab_b.reshape(NPASS, 128, JC, 128).transpose(3, 2, 0, 1)
    return t.reshape(128, 2, 2, NPASS, 128)


def kernel(x: np.ndarray, coeffs: np.ndarray) -> np.ndarray:
    assert x.shape == (8, 2048, IN_F) and coeffs.shape == (OUT_F, IN_F, 12)
    t = np.linspace(0.0, 1.0, 10, dtype=np.float32)  # same knots as reference

    # Segment index via the same float32 comparisons the reference uses.
    xf = np.ascontiguousarray(x.reshape(-1, IN_F))          # [16384, 512]
    seg = np.zeros(xf.shape, dtype=np.int32)
    for m in range(1, 9):
        seg += (xf >= t[m]).astype(np.int32)
    segc = (seg - 4).astype(np.float32)                      # exact in e4m3

    # f[i,j,m] = c_m + c_{m+1} + c_{m+2}; affine fit over {1, m-4, (m-4)^2},
    # residual quantized to e4m3 at global scale a, fit refit on the leftover
    # so the smooth planes absorb the quantization bias.
    c = coeffs.astype(np.float64)
    F = np.stack([c[:, :, m] + c[:, :, m + 1] + c[:, :, m + 2] for m in range(9)])
    F = F.reshape(9, -1)                                     # [9, OUT*IN]
    mc = np.arange(9.0) - 4.0
    Phi = np.stack([np.ones(9), mc, mc * mc], axis=1)        # [9, 3]
    P = np.linalg.pinv(Phi)                                  # [3, 9]
    r = F - Phi @ (P @ F)
    alpha = 240.0 / np.abs(r).max()
    R8b = _q8(r, alpha)                                      # [9, OUT*IN] bytes
    co = P @ (F - R8b.astype(np.float64) / alpha)
    c0, B1, B2 = (co[k].reshape(OUT_F, IN_F) for k in range(3))

    def hi_lo(B):
        hb = _q8(B, alpha)
        lb = _q8(B - hb.astype(np.float64) / alpha, alpha)
        return hb, lb

    B1h, B1l = hi_lo(B1)
    B2h, B2l = hi_lo(B2)

    g_dev = np.empty((128, NU, 2, NPASS, 128), dtype=E4NP)
    for u0, tab in ((0, B1h), (2, B1l), (22, B2h), (24, B2l)):
        pk = _pack_pair(tab)
        for q in range(2):
            g_dev[:, u0 + q] = pk[:, q]
    for m in range(9):
        pk = _pack_pair(R8b[m].reshape(OUT_F, IN_F))
        for q in range(2):
            g_dev[:, 4 + 2 * m + q] = pk[:, q]
    g_dev = np.ascontiguousarray(g_dev)

    base = c0.sum(axis=1)                                    # [OUT] exact f32
    sb = np.empty((128, 1 + NPASS), dtype=np.float32)
    sb[:, 0] = np.float32(1.0 / alpha)
    for ob in range(NPASS):
        sb[:, 1 + ob] = base[ob * 128:(ob + 1) * 128]

    segc_dev = _plane_dev(segc.astype(E4NP))                 # [128, JC, 16384]
    ohp_dev = np.stack(
        [_plane_dev((seg == m).astype(E4NP)) for m in SHIP], axis=1
    )                                                        # [128, S, JC, 16384]

    in_maps = []
    for core in range(N_CORES):
        sl = slice(core * TOK, (core + 1) * TOK)
        in_maps.append(
            {
                "segc": np.ascontiguousarray(segc_dev[:, :, sl]),
                "ohp": np.ascontiguousarray(ohp_dev[:, :, :, sl]),
                "g": g_dev,
                "sb": sb,
            }
        )

    nc = _get_program()
    res = run_bass_kernel_spmd(nc, in_maps, core_ids=list(range(N_CORES)))
    out = np.stack(
        [
            res.results[core]["out"].reshape(OUT_F, TOK).T
            for core in range(N_CORES)
        ]
    )
    return np.ascontiguousarray(out.astype(np.float32))


# revision 6
# speedup vs baseline: 1.8734x; 1.8734x over previous
"""Trainium2 Bass kernel for nn_KANLayer (piecewise-constant KAN forward).

Math: reference computes out[t,i] = sum_j f[i,j,m(x_tj)] where m = segment(x)
in 0..8 and f[i,j,m] = c_m + c_{m+1} + c_{m+2} (9-valued selection -> exact
rank 8 + constant, K = 4096 minimal bf16 contraction = 512 MMs at 216ns).

This kernel runs the whole contraction in fp8-e4m3 DoubleRow (2 fp8 weights
per PE cell -> K=256 per matmul at the same 216ns N=512 stream), with the
table split to keep e4m3 quantization noise inside the 2e-2 budget:

    out[t,i] = base_i + (1/a_i) * [ sum_{m!=4} R8[i,j,m] * onehot_m(t,j)   16 units
                                  + (B1hi+B1lo)[i,j] * (m_tj - 4)           4 units
                                  + (B2hi+B2lo)[i,j] * (m_tj - 4)^2 ]       4 units

R8 = e4m3(a_i * residual) of a per-(i,j) CONSTRAINED affine fit over
{1, m-4, (m-4)^2} with residual(m=4) forced to 0 (drops the m=4 plane and its
2 units); affine coefficients ride hi+lo e4m3 planes (~9-bit mantissa); a_i is
a per-output-row scale applied at evacuation via an AP scalar. All selection
plane values (0/1, m-4, (m-4)^2) are fp8-exact; host does every fit/rounding
and ships raw e4m3 bytes, so device noise is exactly the host-simulated
1.6e-2 on the reference seed.

Structure per core (24 units x 4 out-blocks x 4 token-groups = 384 DR MMs at
216ns = 83us PE): out-blocks are processed in pairs with the UNIT loop outer
(for pair: for u: for ob: for tg) so each unit's table/plane DMA deadline is
~1.7us*u — the full 13MB input stream (planes are all host-shipped; device
builds proved 10-30x slower than DVE bf16 rates) fits at ~358 GB/s. The fp8
table pair is the stationary operand, reused across 4 N=512 matmuls so the
256-col DR LDWEIGHTS (135ns) hides. PSUM: two 4-bank [128, 2048] tiles per
pair. Output leaves as [out_block, 128i, tok] bf16 and is upcast/transposed
on host.

Sharding: data-parallel over tokens, 2048 per core; tables replicated.
"""

from contextlib import ExitStack

import numpy as np

import concourse.bass as bass  # noqa: F401
import concourse.tile as tile
from concourse import bacc, mybir
from concourse.bass_utils import run_bass_kernel_spmd

N_CORES = 8
TOK = 2048          # tokens per core
IN_F = 512
OUT_F = 512
JC = IN_F // 128    # 4 j-chunks of 128
NPASS = OUT_F // 128  # 4 out-blocks
NTG = 4             # token groups (N=512 matmuls) per psum tile
TGW = TOK // NTG
NU = 24             # DR units: 4 lin + 16 onehot (m!=4) + 4 quad
FP8 = mybir.dt.float8e4
BF16 = mybir.dt.bfloat16
F32 = mybir.dt.float32
E4NP = mybir.dt.np(FP8)  # ml_dtypes.float8_e4m3 (TRN: bias 7, max 240)

OH_MS = [0, 1, 2, 3, 5, 6, 7, 8]   # shipped one-hot planes (m=4 dropped)

# unit -> (plane index, jc-pair q). plane tensors: 0=segc (m-4), 1..8=onehot
# for OH_MS, 9=qp ((m-4)^2). lin (hi, lo) first, onehot ascending, quad last.
_UNITS = []
for part in range(2):
    for q in range(2):
        _UNITS.append((0, q))
for k in range(8):
    for q in range(2):
        _UNITS.append((1 + k, q))
for part in range(2):
    for q in range(2):
        _UNITS.append((9, q))
assert len(_UNITS) == NU

_PROGRAM_CACHE = {}


def _build_program():
    nc = bacc.Bacc("TRN2", target_bir_lowering=False, debug=False)

    pl_d = nc.dram_tensor("pl", [128, 10, JC, TOK], FP8, kind="ExternalInput").ap()
    g_d = nc.dram_tensor("g", [128, NU, 2, NPASS, 128], FP8, kind="ExternalInput").ap()
    sb_d = nc.dram_tensor("sb", [128, 2 * NPASS], F32, kind="ExternalInput").ap()
    out_d = nc.dram_tensor("out", [NPASS, 128, TOK], BF16, kind="ExternalOutput").ap()

    with tile.TileContext(nc) as tc, ExitStack() as ctx:
        wm_pool = ctx.enter_context(tc.tile_pool(name="wm", bufs=1))
        pl_pool = ctx.enter_context(tc.tile_pool(name="pl", bufs=1))
        g_pool = ctx.enter_context(tc.tile_pool(name="g", bufs=1))
        sb_pool = ctx.enter_context(tc.tile_pool(name="sb", bufs=1))
        out_pool = ctx.enter_context(tc.tile_pool(name="out", bufs=2))
        psum_pool = ctx.enter_context(tc.tile_pool(name="psum", bufs=2, space="PSUM"))

        # PE warmup on a zeroed scratch tile (HAM un-throttle before real MMs);
        # targets the first psum tile, whose start=True group re-clears it.
        wm = wm_pool.tile([128, 384], BF16, name="wm")
        nc.vector.memset(wm[:], 0.0)

        # --- input DMAs, deadline-ordered across the two HWDGE rings.
        # unit u is first consumed ~8*216ns*u into the kernel; pieces are
        # enqueued in that order, planes on sync, g-table pieces on scalar.
        pl_t = pl_pool.tile([128, 10, JC, TOK], FP8, name="pl")
        g_t = g_pool.tile([128, NU, 2, NPASS, 128], FP8, name="g")
        nc.sync.dma_start(pl_t[:, 0], pl_d[:, 0])              # segc (lin plane)
        g_cuts = [0, 4, 6, 8, 10, 12, 14, 16, 18, 20, 24]
        nc.scalar.dma_start(g_t[:, 0:4], g_d[:, 0:4])
        for k in range(8):                                     # onehot planes
            nc.sync.dma_start(pl_t[:, 1 + k], pl_d[:, 1 + k])
            a, b = g_cuts[k + 1], g_cuts[k + 2]
            nc.scalar.dma_start(g_t[:, a:b], g_d[:, a:b])
        nc.sync.dma_start(pl_t[:, 9], pl_d[:, 9])              # quad plane
        nc.scalar.dma_start(g_t[:, 20:24], g_d[:, 20:24])
        sb_t = sb_pool.tile([128, 2 * NPASS], F32, name="sb")
        nc.gpsimd.dma_start(sb_t[:], sb_d[:])

        # --- main: out-block pairs; unit-outer loop so early units only need
        # the first table/plane pieces while the rest stream in.
        for pair in range(NPASS // 2):
            obs = (2 * pair, 2 * pair + 1)
            pss = {}
            for ob in obs:
                pss[ob] = psum_pool.tile([128, TOK], F32, name="ps")
            if pair == 0:
                for _ in range(20):
                    nc.tensor.matmul(
                        pss[obs[0]][:, :256], wm[:, :128], wm[:, 128:384],
                        start=True, stop=True, skip_group_check=True,
                    )
            for u, (pk, q) in enumerate(_UNITS):
                for ob in obs:
                    lhsT = g_t[:, u, :, ob, :]
                    for tg in range(NTG):
                        nc.tensor.matmul(
                            pss[ob][:, tg * TGW:(tg + 1) * TGW],
                            lhsT,
                            pl_t[:, pk, 2 * q:2 * q + 2, tg * TGW:(tg + 1) * TGW],
                            start=(u == 0),
                            stop=(u == NU - 1),
                            perf_mode=mybir.MatmulPerfMode.DoubleRow,
                        )
            for ob in obs:
                ot = out_pool.tile([128, TOK], BF16, name="ot")
                nc.vector.tensor_scalar(
                    ot[:], pss[ob][:], sb_t[:, ob:ob + 1],
                    sb_t[:, NPASS + ob:NPASS + ob + 1],
                    mybir.AluOpType.mult, mybir.AluOpType.add,
                )
                eng = nc.sync if ob % 2 == 0 else nc.scalar
                eng.dma_start(out_d[ob], ot[:])

    nc.compile()
    return nc


def _get_program():
    if "nc" not in _PROGRAM_CACHE:
        _PROGRAM_CACHE["nc"] = _build_program()
    return _PROGRAM_CACHE["nc"]


def _plane_dev(arr):
    """[T_all, IN] -> [128, JC, T_all] device layout (j = jc*128 + p)."""
    return np.ascontiguousarray(arr.T.reshape(JC, 128, -1).transpose(1, 0, 2))


def _pack_pair(tab_b):
    """e4m3 [OUT, IN] -> [128p, 2q, 2e, NPASS, 128col] stationary layout."""
    t = tab_b.reshape(NPASS, 128, JC, 128).transpose(3, 2, 0, 1)
    return np.ascontiguousarray(t.reshape(128, 2, 2, NPASS, 128))


def kernel(x: np.ndarray, coeffs: np.ndarray) -> np.ndarray:
    assert x.shape == (8, 2048, IN_F) and coeffs.shape == (OUT_F, IN_F, 12)
    t = np.linspace(0.0, 1.0, 10, dtype=np.float32)  # same knots as reference

    # Segment index via the same float32 comparisons the reference uses.
    xf = np.ascontiguousarray(x.reshape(-1, IN_F))          # [16384, 512]
    seg = np.zeros(xf.shape, dtype=np.int32)
    for m in range(1, 9):
        seg += (xf >= t[m]).astype(np.int32)

    # f[m, i, j]; constrained affine fit (residual at m=4 forced to 0).
    c = coeffs.astype(np.float64)
    F = np.stack(
        [c[:, :, m] + c[:, :, m + 1] + c[:, :, m + 2] for m in range(9)]
    ).reshape(9, -1)                                         # [9, OUT*IN]
    mc = np.arange(9.0) - 4.0
    Phi = np.stack([np.ones(9), mc, mc * mc], axis=1)        # [9, 3]
    A = Phi.T @ Phi
    K = np.block([[A, Phi[4:5].T], [Phi[4:5], np.zeros((1, 1))]])
    Kinv = np.linalg.inv(K)

    def fit(Fv):
        return (Kinv @ np.vstack([Phi.T @ Fv, Fv[4:5]]))[:3]

    r = (F - Phi @ fit(F)).reshape(9, OUT_F, IN_F)
    alpha = 240.0 / np.abs(r).max(axis=(0, 2))               # per-out-row scale
    al3 = alpha[None, :, None]

    def q8(v, a):
        return np.clip(v * a, -240.0, 240.0).astype(E4NP)

    R8b = q8(r, al3)                                         # [9, OUT, IN] bytes
    R8 = R8b.astype(np.float64) / al3
    R8[4] = 0.0
    co = fit((F - R8.reshape(9, -1)))
    c0, B1, B2 = (co[k].reshape(OUT_F, IN_F) for k in range(3))

    def hi_lo(B):
        a2 = alpha[:, None]
        hb = q8(B, a2)
        lb = q8(B - hb.astype(np.float64) / a2, a2)
        return hb, lb

    B1h, B1l = hi_lo(B1)
    B2h, B2l = hi_lo(B2)

    g_dev = np.empty((128, NU, 2, NPASS, 128), dtype=E4NP)
    for u0, tab in ((0, B1h), (2, B1l), (20, B2h), (22, B2l)):
        pk = _pack_pair(tab)
        for q in range(2):
            g_dev[:, u0 + q] = pk[:, q]
    for k, m in enumerate(OH_MS):
        pk = _pack_pair(R8b[m])
        for q in range(2):
            g_dev[:, 4 + 2 * k + q] = pk[:, q]
    g_dev = np.ascontiguousarray(g_dev)

    base = c0.sum(axis=1)                                    # [OUT] exact f32
    sb = np.empty((128, 2 * NPASS), dtype=np.float32)
    for ob in range(NPASS):
        sl = slice(ob * 128, (ob + 1) * 128)
        sb[:, ob] = (1.0 / alpha[sl]).astype(np.float32)
        sb[:, NPASS + ob] = base[sl]

    # Plane bytes via uint8 LUTs over seg (much faster than ml_dtypes casts).
    mcv = np.arange(9, dtype=np.float64) - 4.0
    planes = np.empty((128, 10, JC, seg.shape[0]), dtype=E4NP)
    lut_segc = mcv.astype(E4NP).view(np.uint8)
    planes[:, 0] = _plane_dev(lut_segc[seg]).view(E4NP)
    for k, m in enumerate(OH_MS):
        lut = np.zeros(9, E4NP)
        lut[m] = 1.0
        planes[:, 1 + k] = _plane_dev(lut.view(np.uint8)[seg]).view(E4NP)
    lut_qp = (mcv * mcv).astype(E4NP).view(np.uint8)
    planes[:, 9] = _plane_dev(lut_qp[seg]).view(E4NP)

    in_maps = []
    for core in range(N_CORES):
        sl = slice(core * TOK, (core + 1) * TOK)
        in_maps.append(
            {
                "pl": np.ascontiguousarray(planes[:, :, :, sl]),
                "g": g_dev,
                "sb": sb,
            }
        )

    nc = _get_program()
    res = run_bass_kernel_spmd(nc, in_maps, core_ids=list(range(N_CORES)))
    out = np.stack(
        [
            res.results[core]["out"].reshape(OUT_F, TOK).T.astype(np.float32)
            for core in range(N_CORES)
        ]
    )
    return np.ascontiguousarray(out)


# revision 9
# speedup vs baseline: 1.9569x; 1.0446x over previous
"""Trainium2 Bass kernel for nn_KANLayer (piecewise-constant KAN forward).

Math: reference computes out[t,i] = sum_j f[i,j,m(x_tj)] where m = segment(x)
in 0..8 and f[i,j,m] = c_m + c_{m+1} + c_{m+2} (9-valued selection -> exact
rank 8 + constant, K = 4096 minimal bf16 contraction = 512 MMs at 216ns).

This kernel runs the whole contraction in fp8-e4m3 DoubleRow (2 fp8 weights
per PE cell -> K=256 per matmul at the same 216ns N=512 stream), with the
table split to keep e4m3 quantization noise inside the 2e-2 budget:

    out[t,i] = base_i + (1/a_i) * [ sum_{m!=4} R8[i,j,m] * onehot_m(t,j)   16 units
                                  + (B1hi+B1lo)[i,j] * (m_tj - 4)           4 units
                                  + (B2hi+B2lo)[i,j] * (m_tj - 4)^2 ]       4 units

R8 = e4m3(a_i * residual) of a per-(i,j) CONSTRAINED affine fit over
{1, m-4, (m-4)^2} with residual(m=4) forced to 0 (drops the m=4 plane and its
2 units); affine coefficients ride hi+lo e4m3 planes (~9-bit mantissa); a_i is
a per-output-row scale applied at evacuation via an AP scalar. All selection
plane values (0/1, m-4, (m-4)^2) are fp8-exact; host does every fit/rounding
and ships raw e4m3 bytes, so device noise is exactly the host-simulated
1.6e-2 on the reference seed.

Structure per core (24 units x 4 out-blocks x 4 token-groups = 384 DR MMs at
216ns = 83us PE): out-blocks are processed in pairs with the UNIT loop outer
(for pair: for u: for ob: for tg) so each unit's table/plane DMA deadline is
~1.7us*u — the full 13MB input stream (planes are all host-shipped; device
builds proved 10-30x slower than DVE bf16 rates) fits at ~358 GB/s. The fp8
table pair is the stationary operand, reused across 4 N=512 matmuls so the
256-col DR LDWEIGHTS (135ns) hides. PSUM: two 4-bank [128, 2048] tiles per
pair. Output leaves as [out_block, 128i, tok] bf16 and is upcast/transposed
on host.

Sharding: data-parallel over tokens, 2048 per core; tables replicated.
"""

from contextlib import ExitStack

import numpy as np

import concourse.bass as bass  # noqa: F401
import concourse.tile as tile
from concourse import bacc, mybir
from concourse.bass_utils import run_bass_kernel_spmd

N_CORES = 8
TOK = 2048          # tokens per core
IN_F = 512
OUT_F = 512
JC = IN_F // 128    # 4 j-chunks of 128
NPASS = OUT_F // 128  # 4 out-blocks
NTG = 4             # token groups (N=512 matmuls) per psum tile
TGW = TOK // NTG
NU = 24             # DR units: 4 lin + 16 onehot (m!=4) + 4 quad
FP8 = mybir.dt.float8e4
BF16 = mybir.dt.bfloat16
F32 = mybir.dt.float32
E4NP = mybir.dt.np(FP8)  # ml_dtypes.float8_e4m3 (TRN: bias 7, max 240)

OH_MS = [0, 1, 2, 3, 5, 6, 7, 8]   # shipped one-hot planes (m=4 dropped)

# unit -> (plane index, jc-pair q). plane tensors: 0=segc (m-4), 1..8=onehot
# for OH_MS, 9=qp ((m-4)^2). lin (hi, lo) first, onehot ascending, quad last.
_UNITS = []
for part in range(2):
    for q in range(2):
        _UNITS.append((0, q))
for k in range(8):
    for q in range(2):
        _UNITS.append((1 + k, q))
for part in range(2):
    for q in range(2):
        _UNITS.append((9, q))
assert len(_UNITS) == NU

_PROGRAM_CACHE = {}


def _build_program():
    nc = bacc.Bacc("TRN2", target_bir_lowering=False, debug=False)

    pl_d = nc.dram_tensor("pl", [128, 10, JC, TOK], FP8, kind="ExternalInput").ap()
    g_d = nc.dram_tensor("g", [128, NU, 2, NPASS, 128], FP8, kind="ExternalInput").ap()
    sb_d = nc.dram_tensor("sb", [128, 2 * NPASS], F32, kind="ExternalInput").ap()
    out_d = nc.dram_tensor("out", [NPASS, 128, TOK], BF16, kind="ExternalOutput").ap()

    with tile.TileContext(nc) as tc, ExitStack() as ctx:
        wm_pool = ctx.enter_context(tc.tile_pool(name="wm", bufs=1))
        pl_pool = ctx.enter_context(tc.tile_pool(name="pl", bufs=1))
        g_pool = ctx.enter_context(tc.tile_pool(name="g", bufs=1))
        sb_pool = ctx.enter_context(tc.tile_pool(name="sb", bufs=1))
        out_pool = ctx.enter_context(tc.tile_pool(name="out", bufs=2))
        psum_pool = ctx.enter_context(tc.tile_pool(name="psum", bufs=2, space="PSUM"))

        # PE warmup on a zeroed scratch tile (HAM un-throttle before real MMs);
        # targets the first psum tile, whose start=True group re-clears it.
        wm = wm_pool.tile([128, 384], BF16, name="wm")
        nc.vector.memset(wm[:], 0.0)

        # --- input DMAs, deadline-ordered across the two HWDGE rings.
        # unit u is first consumed ~8*216ns*u into the kernel; pieces are
        # enqueued in that order, planes on sync, g-table pieces on scalar.
        pl_t = pl_pool.tile([128, 10, JC, TOK], FP8, name="pl")
        g_t = g_pool.tile([128, NU, 2, NPASS, 128], FP8, name="g")
        for jc in range(JC):                                   # segc (lin plane)
            nc.sync.dma_start(pl_t[:, 0, jc], pl_d[:, 0, jc])
        g_cuts = [0, 4, 6, 8, 10, 12, 14, 16, 18, 20, 24]
        nc.scalar.dma_start(g_t[:, 0:4], g_d[:, 0:4])
        for k in range(8):                                     # onehot planes
            nc.sync.dma_start(pl_t[:, 1 + k], pl_d[:, 1 + k])
            a, b = g_cuts[k + 1], g_cuts[k + 2]
            nc.scalar.dma_start(g_t[:, a:b], g_d[:, a:b])
        nc.sync.dma_start(pl_t[:, 9], pl_d[:, 9])              # quad plane
        nc.scalar.dma_start(g_t[:, 20:24], g_d[:, 20:24])
        sb_t = sb_pool.tile([128, 2 * NPASS], F32, name="sb")
        nc.gpsimd.dma_start(sb_t[:], sb_d[:])

        # --- main loop. 384 DR MMs at 216ns.
        def mm(ps, ob, u, tg):
            pk, q = _UNITS[u]
            nc.tensor.matmul(
                ps[:, tg * TGW:(tg + 1) * TGW],
                g_t[:, u, :, ob, :],
                pl_t[:, pk, 2 * q:2 * q + 2, tg * TGW:(tg + 1) * TGW],
                start=(u == 0),
                stop=(u == NU - 1),
                perf_mode=mybir.MatmulPerfMode.DoubleRow,
            )

        def evac(ps, ob, sl):
            ot = out_pool.tile([128, TOK], BF16, name="ot")
            nc.vector.tensor_scalar(
                ot[:, sl], ps[:, sl], sb_t[:, ob:ob + 1],
                sb_t[:, NPASS + ob:NPASS + ob + 1],
                mybir.AluOpType.mult, mybir.AluOpType.add,
            )
            eng = nc.sync if ob % 2 == 0 else nc.scalar
            eng.dma_start(out_d[ob][:, sl], ot[:, sl])

        # pair 0 (ob 0,1): unit-outer interleave — unit u's table/plane DMA
        # deadline is ~1.7us*u, which the ~13MB input stream meets. ob0 leads
        # ob1 by SKEW units so its psum evacuates while ob1 still streams.
        SKEW = 3
        ps0 = psum_pool.tile([128, TOK], F32, name="ps")
        ps1 = psum_pool.tile([128, TOK], F32, name="ps")
        for _ in range(30):
            nc.tensor.matmul(
                ps0[:, :256], wm[:, :128], wm[:, 128:384],
                start=True, stop=True, skip_group_check=True,
            )
        sched = [(ps0, 0, u) for u in range(SKEW)]
        for u in range(NU):
            sched.append((ps1, 1, u))
            if u + SKEW < NU:
                sched.append((ps0, 0, u + SKEW))
        for ps, ob, u in sched:
            for tg in range(NTG):
                mm(ps, ob, u, tg)
            if u == NU - 1:
                evac(ps, ob, slice(0, TOK))

        # pair 1 (ob 2,3): all inputs resident now — token-group-outer so each
        # [128, 512] psum slice completes early and output trickles out; the
        # exposed tail is just the last slice's evac + DMA.
        for ob in (2, 3):
            ps = psum_pool.tile([128, TOK], F32, name="ps")
            for tg in range(NTG):
                for u in range(NU):
                    mm(ps, ob, u, tg)
                evac(ps, ob, slice(tg * TGW, (tg + 1) * TGW))

    nc.compile()
    return nc


def _get_program():
    if "nc" not in _PROGRAM_CACHE:
        _PROGRAM_CACHE["nc"] = _build_program()
    return _PROGRAM_CACHE["nc"]


def _plane_dev(arr):
    """[T_all, IN] -> [128, JC, T_all] device layout (j = jc*128 + p)."""
    return np.ascontiguousarray(arr.T.reshape(JC, 128, -1).transpose(1, 0, 2))


def _pack_pair(tab_b):
    """e4m3 [OUT, IN] -> [128p, 2q, 2e, NPASS, 128col] stationary layout."""
    t = tab_b.reshape(NPASS, 128, JC, 128).transpose(3, 2, 0, 1)
    return np.ascontiguousarray(t.reshape(128, 2, 2, NPASS, 128))


def kernel(x: np.ndarray, coeffs: np.ndarray) -> np.ndarray:
    assert x.shape == (8, 2048, IN_F) and coeffs.shape == (OUT_F, IN_F, 12)
    t = np.linspace(0.0, 1.0, 10, dtype=np.float32)  # same knots as reference

    # Segment index via the same float32 comparisons the reference uses.
    xf = np.ascontiguousarray(x.reshape(-1, IN_F))          # [16384, 512]
    seg = np.zeros(xf.shape, dtype=np.int32)
    for m in range(1, 9):
        seg += (xf >= t[m]).astype(np.int32)

    # f[m, i, j]; constrained affine fit (residual at m=4 forced to 0).
    c = coeffs.astype(np.float64)
    F = np.stack(
        [c[:, :, m] + c[:, :, m + 1] + c[:, :, m + 2] for m in range(9)]
    ).reshape(9, -1)                                         # [9, OUT*IN]
    mc = np.arange(9.0) - 4.0
    Phi = np.stack([np.ones(9), mc, mc * mc], axis=1)        # [9, 3]
    A = Phi.T @ Phi
    K = np.block([[A, Phi[4:5].T], [Phi[4:5], np.zeros((1, 1))]])
    Kinv = np.linalg.inv(K)

    def fit(Fv):
        return (Kinv @ np.vstack([Phi.T @ Fv, Fv[4:5]]))[:3]

    r = (F - Phi @ fit(F)).reshape(9, OUT_F, IN_F)
    alpha = 240.0 / np.abs(r).max(axis=(0, 2))               # per-out-row scale
    al3 = alpha[None, :, None]

    def q8(v, a):
        return np.clip(v * a, -240.0, 240.0).astype(E4NP)

    R8b = q8(r, al3)                                         # [9, OUT, IN] bytes
    R8 = R8b.astype(np.float64) / al3
    R8[4] = 0.0
    co = fit((F - R8.reshape(9, -1)))
    c0, B1, B2 = (co[k].reshape(OUT_F, IN_F) for k in range(3))

    def hi_lo(B):
        a2 = alpha[:, None]
        hb = q8(B, a2)
        lb = q8(B - hb.astype(np.float64) / a2, a2)
        return hb, lb

    B1h, B1l = hi_lo(B1)
    B2h, B2l = hi_lo(B2)

    g_dev = np.empty((128, NU, 2, NPASS, 128), dtype=E4NP)
    for u0, tab in ((0, B1h), (2, B1l), (20, B2h), (22, B2l)):
        pk = _pack_pair(tab)
        for q in range(2):
            g_dev[:, u0 + q] = pk[:, q]
    for k, m in enumerate(OH_MS):
        pk = _pack_pair(R8b[m])
        for q in range(2):
            g_dev[:, 4 + 2 * k + q] = pk[:, q]
    g_dev = np.ascontiguousarray(g_dev)

    base = c0.sum(axis=1)                                    # [OUT] exact f32
    sb = np.empty((128, 2 * NPASS), dtype=np.float32)
    for ob in range(NPASS):
        sl = slice(ob * 128, (ob + 1) * 128)
        sb[:, ob] = (1.0 / alpha[sl]).astype(np.float32)
        sb[:, NPASS + ob] = base[sl]

    # Plane bytes via uint8 LUTs over seg (much faster than ml_dtypes casts).
    mcv = np.arange(9, dtype=np.float64) - 4.0
    planes = np.empty((128, 10, JC, seg.shape[0]), dtype=E4NP)
    lut_segc = mcv.astype(E4NP).view(np.uint8)
    planes[:, 0] = _plane_dev(lut_segc[seg]).view(E4NP)
    for k, m in enumerate(OH_MS):
        lut = np.zeros(9, E4NP)
        lut[m] = 1.0
        planes[:, 1 + k] = _plane_dev(lut.view(np.uint8)[seg]).view(E4NP)
    lut_qp = (mcv * mcv).astype(E4NP).view(np.uint8)
    planes[:, 9] = _plane_dev(lut_qp[seg]).view(E4NP)

    in_maps = []
    for core in range(N_CORES):
        sl = slice(core * TOK, (core + 1) * TOK)
        in_maps.append(
            {
                "pl": np.ascontiguousarray(planes[:, :, :, sl]),
                "g": g_dev,
                "sb": sb,
            }
        )

    nc = _get_program()
    res = run_bass_kernel_spmd(nc, in_maps, core_ids=list(range(N_CORES)))
    out = np.stack(
        [
            res.results[core]["out"].reshape(OUT_F, TOK).T.astype(np.float32)
            for core in range(N_CORES)
        ]
    )
    return np.ascontiguousarray(out)


# revision 10
# speedup vs baseline: 2.1157x; 1.0811x over previous
"""Trainium2 Bass kernel for nn_KANLayer (piecewise-constant KAN forward).

Math: reference computes out[t,i] = sum_j f[i,j,m(x_tj)] where m = segment(x)
in 0..8 and f[i,j,m] = c_m + c_{m+1} + c_{m+2} (9-valued selection -> exact
rank 8 + constant; the bf16 version needs K=4096 = 512 MMs at 216ns/core).

This kernel runs the whole contraction in fp8-e4m3 DoubleRow (2 fp8 weights
per PE cell -> K=256 per matmul at the same 216ns N=512 stream = 2x bf16
FLOPs), with the table split to keep e4m3 quantization noise in budget:

    out[t,i] = base_i + (1/a_i) * [ sum_{m!=4} R8[i,j,m] * onehot_m(t,j)  16 units
                                  + (B1hi+B1lo)[i,j] * (m_tj-4)            4 units
                                  + B2q[i,j] * (m_tj-4)^2 ]                2 units

Table construction (host, f64) exploits quantization-error absorption:
c0 is pinned to f(4) (residual at m=4 is exactly zero -> the m=4 one-hot
plane and its 2 units are dropped); B2 is quantized FIRST so its e4m3 error
is absorbed into the later-quantized one-hot residual R8; B1 is refit LAST on
the leftover (absorbing the m-linear component of R8's quantization noise)
and carried hi+lo (~9-bit). a_i is a per-output-row scale applied at
evacuation via an AP scalar. All plane values (0/1, m-4, (m-4)^2) are
fp8-exact; host ships raw e4m3 bytes. End-to-end noise on the reference
seed: 1.7e-2 (threshold 2e-2), verified by exact host simulation.

Structure per core: 22 units x 4 out-blocks x 4 token-groups = 352 DR MMs at
216ns = 76us PE. PSUM is 8 single-bank [128,512] tiles so each token-group
slice's evacuation (DVE scale+bias -> bf16 -> DMA) never blocks the next
slice's accumulation. Pair 0 (out-blocks 0,1) runs unit-outer with ob0
skewed 3 units ahead (DMA-deadline-friendly while tables/planes stream in,
evacs hidden); pair 1 runs token-group-outer so the exposed tail is one
slice. The fp8 table pair is stationary, reused across 4 N=512 matmuls, so
the 256-col DR LDWEIGHTS (135ns) hides. The (m-4)^2 plane is squared from
the (m-4) plane on the otherwise-idle ACT engine; everything else is
host-shipped (device-side fp8 DVE/GpSimd builds measured 10-30x slower than
bf16 rates). Output leaves as [out_block, 128i, tok] bf16, upcast/transposed
on host. Sharding: data-parallel over tokens, 2048 per core; tables
replicated.
"""

from contextlib import ExitStack

import numpy as np

import concourse.bass as bass  # noqa: F401
import concourse.tile as tile
from concourse import bacc, mybir
from concourse.bass_utils import run_bass_kernel_spmd

N_CORES = 8
TOK = 2048          # tokens per core
IN_F = 512
OUT_F = 512
JC = IN_F // 128    # 4 j-chunks of 128
NPASS = OUT_F // 128  # 4 out-blocks
NTG = 4             # token groups (N=512 matmuls) per out-block
TGW = TOK // NTG
NU = 22             # DR units: 4 lin (hi+lo) + 16 onehot (m!=4) + 2 quad
FP8 = mybir.dt.float8e4
BF16 = mybir.dt.bfloat16
F32 = mybir.dt.float32
E4NP = mybir.dt.np(FP8)  # ml_dtypes.float8_e4m3 (TRN: bias 7, max 240)

OH_MS = [0, 1, 2, 3, 5, 6, 7, 8]   # shipped one-hot planes (m=4 dropped)

# unit -> (plane index, jc-pair q). planes: 0=segc (m-4), 1..8=onehot for
# OH_MS, 9=qp ((m-4)^2, ACT-built). lin hi+lo first, onehot ascending, quad
# last (gives ACT time to square).
_UNITS = []
for part in range(2):
    for q in range(2):
        _UNITS.append((0, q))
for k in range(8):
    for q in range(2):
        _UNITS.append((1 + k, q))
for q in range(2):
    _UNITS.append((9, q))
assert len(_UNITS) == NU

_PROGRAM_CACHE = {}


def _build_program():
    nc = bacc.Bacc("TRN2", target_bir_lowering=False, debug=False)

    pl_d = nc.dram_tensor("pl", [128, 9, JC, TOK], FP8, kind="ExternalInput").ap()
    g_d = nc.dram_tensor("g", [128, NU, 2, NPASS, 128], FP8, kind="ExternalInput").ap()
    sb_d = nc.dram_tensor("sb", [128, 2 * NPASS], F32, kind="ExternalInput").ap()
    out_d = nc.dram_tensor("out", [NPASS, 128, TOK], BF16, kind="ExternalOutput").ap()

    with tile.TileContext(nc) as tc, ExitStack() as ctx:
        wm_pool = ctx.enter_context(tc.tile_pool(name="wm", bufs=1))
        pl_pool = ctx.enter_context(tc.tile_pool(name="pl", bufs=1))
        g_pool = ctx.enter_context(tc.tile_pool(name="g", bufs=1))
        sb_pool = ctx.enter_context(tc.tile_pool(name="sb", bufs=1))
        out_pool = ctx.enter_context(tc.tile_pool(name="out", bufs=4))
        psum_pool = ctx.enter_context(tc.tile_pool(name="psum", bufs=8, space="PSUM"))

        wm = wm_pool.tile([128, 384], BF16, name="wm")
        nc.vector.memset(wm[:], 0.0)

        # --- input DMAs, deadline-ordered, planes and g interleaved across
        # the two HWDGE rings. pl layout: slot 0 = segc, 1..8 = one-hots.
        pl_t = pl_pool.tile([128, 10, JC, TOK], FP8, name="pl")
        g_t = g_pool.tile([128, NU, 2, NPASS, 128], FP8, name="g")
        for jc in range(JC):                                   # segc first
            nc.sync.dma_start(pl_t[:, 0, jc], pl_d[:, 0, jc])
        nc.scalar.dma_start(g_t[:, 0:4], g_d[:, 0:4])          # lin hi+lo
        g_cuts = [4, 8, 12, 16, 22]
        for k in range(8):                                     # onehot planes
            eng = nc.sync if k % 2 == 0 else nc.scalar
            eng.dma_start(pl_t[:, 1 + k], pl_d[:, 1 + k])
            if k % 2 == 1:
                a, b = g_cuts[k // 2], g_cuts[k // 2 + 1]
                nc.scalar.dma_start(g_t[:, a:b], g_d[:, a:b])
        sb_t = sb_pool.tile([128, 2 * NPASS], F32, name="sb")
        nc.gpsimd.dma_start(sb_t[:], sb_d[:])

        # quad plane on the ACT engine: (m-4)^2 from segc, exact in e4m3.
        for jc in range(JC):
            nc.scalar.square(pl_t[:, 9, jc], pl_t[:, 0, jc])

        def mm(ps, ob, u, tg):
            pk, q = _UNITS[u]
            nc.tensor.matmul(
                ps,
                g_t[:, u, :, ob, :],
                pl_t[:, pk, 2 * q:2 * q + 2, tg * TGW:(tg + 1) * TGW],
                start=(u == 0),
                stop=(u == NU - 1),
                perf_mode=mybir.MatmulPerfMode.DoubleRow,
            )

        def evac(ps, ob, tg):
            ot = out_pool.tile([128, TGW], BF16, name="ot")
            nc.vector.tensor_scalar(
                ot[:], ps[:], sb_t[:, ob:ob + 1],
                sb_t[:, NPASS + ob:NPASS + ob + 1],
                mybir.AluOpType.mult, mybir.AluOpType.add,
            )
            eng = nc.sync if ob % 2 == 0 else nc.scalar
            eng.dma_start(out_d[ob][:, tg * TGW:(tg + 1) * TGW], ot[:])

        # pair 0 (ob 0,1): unit-outer interleave — unit u's table/plane DMA
        # deadline is ~1.7us*u. ob0 leads ob1 by SKEW units so its psum
        # evacuations overlap ob1's stream.
        SKEW = 3
        pss = {
            ob: [psum_pool.tile([128, TGW], F32, name="ps") for _ in range(NTG)]
            for ob in (0, 1)
        }
        for _ in range(30):
            nc.tensor.matmul(
                pss[0][0][:, :256], wm[:, :128], wm[:, 128:384],
                start=True, stop=True, skip_group_check=True,
            )
        sched = [(0, u) for u in range(SKEW)]
        for u in range(NU):
            sched.append((1, u))
            if u + SKEW < NU:
                sched.append((0, u + SKEW))
        for ob, u in sched:
            for tg in range(NTG):
                mm(pss[ob][tg][:], ob, u, tg)
            if u == NU - 1:
                for tg in range(NTG):
                    evac(pss[ob][tg], ob, tg)

        # pair 1 (ob 2,3): all inputs resident — token-group-outer so each
        # single-bank psum completes early and output trickles out.
        for ob in (2, 3):
            for tg in range(NTG):
                ps = psum_pool.tile([128, TGW], F32, name="ps")
                for u in range(NU):
                    mm(ps[:], ob, u, tg)
                evac(ps, ob, tg)

    nc.compile()
    return nc


def _get_program():
    if "nc" not in _PROGRAM_CACHE:
        _PROGRAM_CACHE["nc"] = _build_program()
    return _PROGRAM_CACHE["nc"]


def _plane_dev(arr):
    """[T_all, IN] -> [128, JC, T_all] device layout (j = jc*128 + p)."""
    return np.ascontiguousarray(arr.T.reshape(JC, 128, -1).transpose(1, 0, 2))


def _pack_pair(tab_b):
    """e4m3 [OUT, IN] -> [128p, 2q, 2e, NPASS, 128col] stationary layout."""
    t = tab_b.reshape(NPASS, 128, JC, 128).transpose(3, 2, 0, 1)
    return np.ascontiguousarray(t.reshape(128, 2, 2, NPASS, 128))


def kernel(x: np.ndarray, coeffs: np.ndarray) -> np.ndarray:
    assert x.shape == (8, 2048, IN_F) and coeffs.shape == (OUT_F, IN_F, 12)
    t = np.linspace(0.0, 1.0, 10, dtype=np.float32)  # same knots as reference

    # Segment index via the same float32 comparisons the reference uses.
    xf = np.ascontiguousarray(x.reshape(-1, IN_F))          # [16384, 512]
    seg = np.zeros(xf.shape, dtype=np.int32)
    for m in range(1, 9):
        seg += (xf >= t[m]).astype(np.int32)

    # Table build (see module docstring): c0 = f(4); B2 quantized first
    # (absorbed); R8 next; B1 refit last, hi+lo.
    c = coeffs.astype(np.float64)
    F = np.stack(
        [c[:, :, m] + c[:, :, m + 1] + c[:, :, m + 2] for m in range(9)]
    ).reshape(9, -1)                                         # [9, OUT*IN]
    mc = np.arange(9.0) - 4.0
    qv = mc * mc
    D = F - F[4:5]
    Phi2 = np.stack([mc, qv], axis=1)                        # [9, 2]
    co = np.linalg.lstsq(Phi2, D, rcond=None)[0]
    r0 = (D - Phi2 @ co).reshape(9, OUT_F, IN_F)
    alpha = 240.0 / (1.02 * np.abs(r0).max(axis=(0, 2)))     # per-out-row
    a2 = alpha[:, None]
    a3 = alpha[None, :, None]

    def q8(v, a):
        return np.clip(v * a, -240.0, 240.0).astype(E4NP)

    B1, B2 = (co[k].reshape(OUT_F, IN_F) for k in range(2))
    B2b = q8(B2, a2)
    B2q = B2b.astype(np.float64) / a2
    res = (
        D.reshape(9, OUT_F, IN_F)
        - B1[None] * mc[:, None, None]
        - B2q[None] * qv[:, None, None]
    )
    R8b = q8(res, a3)
    R8 = R8b.astype(np.float64) / a3
    R8[4] = 0.0
    left = D.reshape(9, OUT_F, IN_F) - B2q[None] * qv[:, None, None] - R8
    B1r = np.einsum("m,mij->ij", mc, left) / (mc @ mc)
    B1hb = q8(B1r, a2)
    B1lb = q8(B1r - B1hb.astype(np.float64) / a2, a2)

    g_dev = np.empty((128, NU, 2, NPASS, 128), dtype=E4NP)
    for u0, tab in ((0, B1hb), (2, B1lb), (20, B2b)):
        pk = _pack_pair(tab)
        for q in range(2):
            g_dev[:, u0 + q] = pk[:, q]
    for k, m in enumerate(OH_MS):
        pk = _pack_pair(R8b[m])
        for q in range(2):
            g_dev[:, 4 + 2 * k + q] = pk[:, q]
    g_dev = np.ascontiguousarray(g_dev)

    base = F[4].reshape(OUT_F, IN_F).sum(axis=1)             # exact fp32
    sb = np.empty((128, 2 * NPASS), dtype=np.float32)
    for ob in range(NPASS):
        sl = slice(ob * 128, (ob + 1) * 128)
        sb[:, ob] = (1.0 / alpha[sl]).astype(np.float32)
        sb[:, NPASS + ob] = base[sl]

    # Plane bytes via uint8 LUTs over seg (fast).
    planes = np.empty((128, 9, JC, seg.shape[0]), dtype=E4NP)
    lut_segc = mc.astype(E4NP).view(np.uint8)
    planes[:, 0] = _plane_dev(lut_segc[seg]).view(E4NP)
    for k, m in enumerate(OH_MS):
        lut = np.zeros(9, E4NP)
        lut[m] = 1.0
        planes[:, 1 + k] = _plane_dev(lut.view(np.uint8)[seg]).view(E4NP)

    in_maps = []
    for core in range(N_CORES):
        sl = slice(core * TOK, (core + 1) * TOK)
        in_maps.append(
            {
                "pl": np.ascontiguousarray(planes[:, :, :, sl]),
                "g": g_dev,
                "sb": sb,
            }
        )

    nc = _get_program()
    res_ = run_bass_kernel_spmd(nc, in_maps, core_ids=list(range(N_CORES)))
    out = np.stack(
        [
            res_.results[core]["out"].reshape(OUT_F, TOK).T.astype(np.float32)
            for core in range(N_CORES)
        ]
    )
    return np.ascontiguousarray(out)


# revision 11
# speedup vs baseline: 2.1218x; 1.0029x over previous
"""Trainium2 Bass kernel for nn_KANLayer (piecewise-constant KAN forward).

Math: reference computes out[t,i] = sum_j f[i,j,m(x_tj)] where m = segment(x)
in 0..8 and f[i,j,m] = c_m + c_{m+1} + c_{m+2} (9-valued selection -> exact
rank 8 + constant; the bf16 version needs K=4096 = 512 MMs at 216ns/core).

This kernel runs the whole contraction in fp8-e4m3 DoubleRow (2 fp8 weights
per PE cell -> K=256 per matmul at the same 216ns N=512 stream = 2x bf16
FLOPs), with the table split to keep e4m3 quantization noise in budget:

    out[t,i] = base_i + (1/a_i) * [ sum_{m!=4} R8[i,j,m] * onehot_m(t,j)  16 units
                                  + B1q[i,j] * (m_tj-4)                    2 units
                                  + B2q[i,j] * (m_tj-4)^2 ]                2 units

Table construction (host, f64) exploits quantization-error absorption:
c0 is pinned to f(4) (residual at m=4 is exactly zero -> the m=4 one-hot
plane and its 2 units are dropped); B1 and B2 are quantized FIRST
(single e4m3 pass each) so their quantization error is absorbed into the
later-quantized one-hot residual R8 — the absorption direction that keeps
total noise at 1.8e-2 with only one pass per affine plane. a_i is a per-output-row scale applied at
evacuation via an AP scalar. All plane values (0/1, m-4, (m-4)^2) are
fp8-exact; host ships raw e4m3 bytes. End-to-end noise on the reference
seed: 1.82e-2 (threshold 2e-2), verified by exact full-set host simulation.

Structure per core: 20 units x 4 out-blocks x 4 token-groups = 320 DR MMs at
216ns = 76us PE. PSUM is 8 single-bank [128,512] tiles so each token-group
slice's evacuation (DVE scale+bias -> bf16 -> DMA) never blocks the next
slice's accumulation. Pair 0 (out-blocks 0,1) runs unit-outer with ob0
skewed 3 units ahead (DMA-deadline-friendly while tables/planes stream in,
evacs hidden); pair 1 runs token-group-outer so the exposed tail is one
slice. The fp8 table pair is stationary, reused across 4 N=512 matmuls, so
the 256-col DR LDWEIGHTS (135ns) hides. The (m-4)^2 plane is squared from
the (m-4) plane on the otherwise-idle ACT engine; everything else is
host-shipped (device-side fp8 DVE/GpSimd builds measured 10-30x slower than
bf16 rates). Output leaves as [out_block, 128i, tok] bf16, upcast/transposed
on host. Sharding: data-parallel over tokens, 2048 per core; tables
replicated.
"""

from contextlib import ExitStack

import numpy as np

import concourse.bass as bass  # noqa: F401
import concourse.tile as tile
from concourse import bacc, mybir
from concourse.bass_utils import run_bass_kernel_spmd

N_CORES = 8
TOK = 2048          # tokens per core
IN_F = 512
OUT_F = 512
JC = IN_F // 128    # 4 j-chunks of 128
NPASS = OUT_F // 128  # 4 out-blocks
NTG = 4             # token groups (N=512 matmuls) per out-block
TGW = TOK // NTG
NU = 20             # DR units: 2 lin + 16 onehot (m!=4) + 2 quad
FP8 = mybir.dt.float8e4
BF16 = mybir.dt.bfloat16
F32 = mybir.dt.float32
E4NP = mybir.dt.np(FP8)  # ml_dtypes.float8_e4m3 (TRN: bias 7, max 240)

OH_MS = [0, 1, 2, 3, 5, 6, 7, 8]   # shipped one-hot planes (m=4 dropped)

# unit -> (plane index, jc-pair q). planes: 0=segc (m-4), 1..8=onehot for
# OH_MS, 9=qp ((m-4)^2, ACT-built). lin hi+lo first, onehot ascending, quad
# last (gives ACT time to square).
_UNITS = []
for q in range(2):
    _UNITS.append((0, q))
for k in range(8):
    for q in range(2):
        _UNITS.append((1 + k, q))
for q in range(2):
    _UNITS.append((9, q))
assert len(_UNITS) == NU

_PROGRAM_CACHE = {}


def _build_program():
    nc = bacc.Bacc("TRN2", target_bir_lowering=False, debug=False)

    pl_d = nc.dram_tensor("pl", [128, 9, JC, TOK], FP8, kind="ExternalInput").ap()
    g_d = nc.dram_tensor("g", [128, NU, 2, NPASS, 128], FP8, kind="ExternalInput").ap()
    sb_d = nc.dram_tensor("sb", [128, 2 * NPASS], F32, kind="ExternalInput").ap()
    out_d = nc.dram_tensor("out", [NPASS, 128, TOK], BF16, kind="ExternalOutput").ap()

    with tile.TileContext(nc) as tc, ExitStack() as ctx:
        wm_pool = ctx.enter_context(tc.tile_pool(name="wm", bufs=1))
        pl_pool = ctx.enter_context(tc.tile_pool(name="pl", bufs=1))
        g_pool = ctx.enter_context(tc.tile_pool(name="g", bufs=1))
        sb_pool = ctx.enter_context(tc.tile_pool(name="sb", bufs=1))
        out_pool = ctx.enter_context(tc.tile_pool(name="out", bufs=4))
        psum_pool = ctx.enter_context(tc.tile_pool(name="psum", bufs=8, space="PSUM"))

        wm = wm_pool.tile([128, 384], BF16, name="wm")
        nc.vector.memset(wm[:], 0.0)

        # --- input DMAs, deadline-ordered, planes and g interleaved across
        # the two HWDGE rings. pl layout: slot 0 = segc, 1..8 = one-hots.
        pl_t = pl_pool.tile([128, 10, JC, TOK], FP8, name="pl")
        g_t = g_pool.tile([128, NU, 2, NPASS, 128], FP8, name="g")
        for jc in range(JC):                                   # segc first
            nc.sync.dma_start(pl_t[:, 0, jc], pl_d[:, 0, jc])
        nc.scalar.dma_start(g_t[:, 0:2], g_d[:, 0:2])          # lin
        g_cuts = [2, 6, 10, 14, 20]
        for k in range(8):                                     # onehot planes
            eng = nc.sync if k % 2 == 0 else nc.scalar
            eng.dma_start(pl_t[:, 1 + k], pl_d[:, 1 + k])
            if k % 2 == 1:
                a, b = g_cuts[k // 2], g_cuts[k // 2 + 1]
                nc.scalar.dma_start(g_t[:, a:b], g_d[:, a:b])
        sb_t = sb_pool.tile([128, 2 * NPASS], F32, name="sb")
        nc.gpsimd.dma_start(sb_t[:], sb_d[:])

        # quad plane on the ACT engine: (m-4)^2 from segc, exact in e4m3.
        for jc in range(JC):
            nc.scalar.square(pl_t[:, 9, jc], pl_t[:, 0, jc])

        def mm(ps, ob, u, tg):
            pk, q = _UNITS[u]
            nc.tensor.matmul(
                ps,
                g_t[:, u, :, ob, :],
                pl_t[:, pk, 2 * q:2 * q + 2, tg * TGW:(tg + 1) * TGW],
                start=(u == 0),
                stop=(u == NU - 1),
                perf_mode=mybir.MatmulPerfMode.DoubleRow,
            )

        def evac(ps, ob, tg):
            ot = out_pool.tile([128, TGW], BF16, name="ot")
            nc.vector.tensor_scalar(
                ot[:], ps[:], sb_t[:, ob:ob + 1],
                sb_t[:, NPASS + ob:NPASS + ob + 1],
                mybir.AluOpType.mult, mybir.AluOpType.add,
            )
            eng = nc.sync if ob % 2 == 0 else nc.scalar
            eng.dma_start(out_d[ob][:, tg * TGW:(tg + 1) * TGW], ot[:])

        # pair 0 (ob 0,1): unit-outer interleave — unit u's table/plane DMA
        # deadline is ~1.7us*u. ob0 leads ob1 by SKEW units so its psum
        # evacuations overlap ob1's stream.
        SKEW = 3
        pss = {
            ob: [psum_pool.tile([128, TGW], F32, name="ps") for _ in range(NTG)]
            for ob in (0, 1)
        }
        for _ in range(76):
            nc.tensor.matmul(
                pss[0][0][:, :256], wm[:, :128], wm[:, 128:384],
                start=True, stop=True, skip_group_check=True,
            )
        sched = [(0, u) for u in range(SKEW)]
        for u in range(NU):
            sched.append((1, u))
            if u + SKEW < NU:
                sched.append((0, u + SKEW))
        for ob, u in sched:
            for tg in range(NTG):
                mm(pss[ob][tg][:], ob, u, tg)
            if u == NU - 1:
                for tg in range(NTG):
                    evac(pss[ob][tg], ob, tg)

        # pair 1 (ob 2,3): all inputs resident — token-group-outer so each
        # single-bank psum completes early and output trickles out.
        for ob in (2, 3):
            for tg in range(NTG):
                ps = psum_pool.tile([128, TGW], F32, name="ps")
                for u in range(NU):
                    mm(ps[:], ob, u, tg)
                evac(ps, ob, tg)

    nc.compile()
    return nc


def _get_program():
    if "nc" not in _PROGRAM_CACHE:
        _PROGRAM_CACHE["nc"] = _build_program()
    return _PROGRAM_CACHE["nc"]


def _plane_dev(arr):
    """[T_all, IN] -> [128, JC, T_all] device layout (j = jc*128 + p)."""
    return np.ascontiguousarray(arr.T.reshape(JC, 128, -1).transpose(1, 0, 2))


def _pack_pair(tab_b):
    """e4m3 [OUT, IN] -> [128p, 2q, 2e, NPASS, 128col] stationary layout."""
    t = tab_b.reshape(NPASS, 128, JC, 128).transpose(3, 2, 0, 1)
    return np.ascontiguousarray(t.reshape(128, 2, 2, NPASS, 128))


def kernel(x: np.ndarray, coeffs: np.ndarray) -> np.ndarray:
    assert x.shape == (8, 2048, IN_F) and coeffs.shape == (OUT_F, IN_F, 12)
    t = np.linspace(0.0, 1.0, 10, dtype=np.float32)  # same knots as reference

    # Segment index via the same float32 comparisons the reference uses.
    xf = np.ascontiguousarray(x.reshape(-1, IN_F))          # [16384, 512]
    seg = np.zeros(xf.shape, dtype=np.int32)
    for m in range(1, 9):
        seg += (xf >= t[m]).astype(np.int32)

    # Table build (see module docstring): c0 = f(4); B2 quantized first
    # (absorbed); R8 next; B1 refit last, hi+lo.
    c = coeffs.astype(np.float64)
    F = np.stack(
        [c[:, :, m] + c[:, :, m + 1] + c[:, :, m + 2] for m in range(9)]
    ).reshape(9, -1)                                         # [9, OUT*IN]
    mc = np.arange(9.0) - 4.0
    qv = mc * mc
    D = F - F[4:5]
    Phi2 = np.stack([mc, qv], axis=1)                        # [9, 2]
    co = np.linalg.lstsq(Phi2, D, rcond=None)[0]
    r0 = (D - Phi2 @ co).reshape(9, OUT_F, IN_F)
    alpha = 240.0 / (1.02 * np.abs(r0).max(axis=(0, 2)))     # per-out-row
    a2 = alpha[:, None]
    a3 = alpha[None, :, None]

    def q8(v, a):
        return np.clip(v * a, -240.0, 240.0).astype(E4NP)

    B1, B2 = (co[k].reshape(OUT_F, IN_F) for k in range(2))
    B1b = q8(B1, a2)
    B2b = q8(B2, a2)
    B1q = B1b.astype(np.float64) / a2
    B2q = B2b.astype(np.float64) / a2
    res = (
        D.reshape(9, OUT_F, IN_F)
        - B1q[None] * mc[:, None, None]
        - B2q[None] * qv[:, None, None]
    )
    R8b = q8(res, a3)
    R8b[4] = 0

    g_dev = np.empty((128, NU, 2, NPASS, 128), dtype=E4NP)
    for u0, tab in ((0, B1b), (18, B2b)):
        pk = _pack_pair(tab)
        for q in range(2):
            g_dev[:, u0 + q] = pk[:, q]
    for k, m in enumerate(OH_MS):
        pk = _pack_pair(R8b[m])
        for q in range(2):
            g_dev[:, 2 + 2 * k + q] = pk[:, q]
    g_dev = np.ascontiguousarray(g_dev)

    base = F[4].reshape(OUT_F, IN_F).sum(axis=1)             # exact fp32
    sb = np.empty((128, 2 * NPASS), dtype=np.float32)
    for ob in range(NPASS):
        sl = slice(ob * 128, (ob + 1) * 128)
        sb[:, ob] = (1.0 / alpha[sl]).astype(np.float32)
        sb[:, NPASS + ob] = base[sl]

    # Plane bytes via uint8 LUTs over seg (fast).
    planes = np.empty((128, 9, JC, seg.shape[0]), dtype=E4NP)
    lut_segc = mc.astype(E4NP).view(np.uint8)
    planes[:, 0] = _plane_dev(lut_segc[seg]).view(E4NP)
    for k, m in enumerate(OH_MS):
        lut = np.zeros(9, E4NP)
        lut[m] = 1.0
        planes[:, 1 + k] = _plane_dev(lut.view(np.uint8)[seg]).view(E4NP)

    in_maps = []
    for core in range(N_CORES):
        sl = slice(core * TOK, (core + 1) * TOK)
        in_maps.append(
            {
                "pl": np.ascontiguousarray(planes[:, :, :, sl]),
                "g": g_dev,
                "sb": sb,
            }
        )

    nc = _get_program()
    res_ = run_bass_kernel_spmd(nc, in_maps, core_ids=list(range(N_CORES)))
    out = np.stack(
        [
            res_.results[core]["out"].reshape(OUT_F, TOK).T.astype(np.float32)
            for core in range(N_CORES)
        ]
    )
    return np.ascontiguousarray(out)


# revision 13
# speedup vs baseline: 2.1634x; 1.0196x over previous
"""Trainium2 Bass kernel for nn_KANLayer (piecewise-constant KAN forward).

Math: reference computes out[t,i] = sum_j f[i,j,m(x_tj)] where m = segment(x)
in 0..8 and f[i,j,m] = c_m + c_{m+1} + c_{m+2} (9-valued selection -> exact
rank 8 + constant; the bf16 version needs K=4096 = 512 MMs at 216ns/core).

This kernel runs the whole contraction in fp8-e4m3 DoubleRow (2 fp8 weights
per PE cell -> K=256 per matmul at the same 216ns N=512 stream = 2x bf16
FLOPs), with the table split to keep e4m3 quantization noise in budget:

    out[t,i] = base_i + (1/a_i) * [ sum_{m!=4} R8[i,j,m] * onehot_m(t,j)  16 units
                                  + B1q[i,j] * (m_tj-4)                    2 units
                                  + B2q[i,j] * (m_tj-4)^2 ]                2 units

Table construction (host, f64) exploits quantization-error absorption:
c0 is pinned to f(4) (residual at m=4 is exactly zero -> the m=4 one-hot
plane and its 2 units are dropped); B1 and B2 are quantized FIRST
(single e4m3 pass each) so their quantization error is absorbed into the
later-quantized one-hot residual R8 — the absorption direction that keeps
total noise at 1.8e-2 with only one pass per affine plane. a_i is a per-output-row scale applied at
evacuation via an AP scalar. All plane values (0/1, m-4, (m-4)^2) are
fp8-exact; host ships raw e4m3 bytes. End-to-end noise on the reference
seed: 1.82e-2 (threshold 2e-2), verified by exact full-set host simulation.

Structure per core: 20 units x 4 out-blocks x 4 token-groups = 320 DR MMs at
216ns = 76us PE. PSUM is 8 single-bank [128,512] tiles so each token-group
slice's evacuation (DVE scale+bias -> bf16 -> DMA) never blocks the next
slice's accumulation. Pair 0 (out-blocks 0,1) runs unit-outer with ob0
skewed 3 units ahead (DMA-deadline-friendly while tables/planes stream in,
evacs hidden); pair 1 runs token-group-outer so the exposed tail is one
slice. The fp8 table pair is stationary, reused across 4 N=512 matmuls, so
the 256-col DR LDWEIGHTS (135ns) hides. The (m-4)^2 plane is squared from
the (m-4) plane on the otherwise-idle ACT engine; everything else is
host-shipped (device-side fp8 DVE/GpSimd builds measured 10-30x slower than
bf16 rates). Output leaves as [out_block, 128i, tok] bf16, upcast/transposed
on host. Sharding: data-parallel over tokens, 2048 per core; tables
replicated.
"""

from contextlib import ExitStack

import numpy as np

import concourse.bass as bass  # noqa: F401
import concourse.tile as tile
from concourse import bacc, mybir
from concourse.bass_utils import run_bass_kernel_spmd

N_CORES = 8
TOK = 2048          # tokens per core
IN_F = 512
OUT_F = 512
JC = IN_F // 128    # 4 j-chunks of 128
NPASS = OUT_F // 128  # 4 out-blocks
NTG = 4             # token groups (N=512 matmuls) per out-block
TGW = TOK // NTG
NU = 20             # DR units: 2 lin + 16 onehot (m!=4) + 2 quad
FP8 = mybir.dt.float8e4
BF16 = mybir.dt.bfloat16
F32 = mybir.dt.float32
E4NP = mybir.dt.np(FP8)  # ml_dtypes.float8_e4m3 (TRN: bias 7, max 240)

OH_MS = [0, 1, 2, 3, 5, 6, 7]      # shipped one-hot planes (m=4 dropped)

# unit -> (plane index, jc-pair q). planes: 0=segc (m-4), 1..7=onehot for
# OH_MS, 8=qp ((m-4)^2, ACT-built), 9=onehot m=8 (ACT-built via
# relu(1-(m-8)^2)). lin first, shipped onehots ascending, ACT-built last.
_UNITS = []
for q in range(2):
    _UNITS.append((0, q))
for k in range(7):
    for q in range(2):
        _UNITS.append((1 + k, q))
for pk in (8, 9):
    for q in range(2):
        _UNITS.append((pk, q))
assert len(_UNITS) == NU

_PROGRAM_CACHE = {}


def _build_program():
    nc = bacc.Bacc("TRN2", target_bir_lowering=False, debug=False)

    pl_d = nc.dram_tensor("pl", [128, 8, JC, TOK], FP8, kind="ExternalInput").ap()
    g_d = nc.dram_tensor("g", [128, NU, 2, NPASS, 128], FP8, kind="ExternalInput").ap()
    sb_d = nc.dram_tensor("sb", [128, 2 * NPASS], F32, kind="ExternalInput").ap()
    out_d = nc.dram_tensor("out", [NPASS, 128, TOK], BF16, kind="ExternalOutput").ap()

    with tile.TileContext(nc) as tc, ExitStack() as ctx:
        wm_pool = ctx.enter_context(tc.tile_pool(name="wm", bufs=1))
        tmp_pool = ctx.enter_context(tc.tile_pool(name="tmp", bufs=1))
        pl_pool = ctx.enter_context(tc.tile_pool(name="pl", bufs=1))
        g_pool = ctx.enter_context(tc.tile_pool(name="g", bufs=1))
        sb_pool = ctx.enter_context(tc.tile_pool(name="sb", bufs=1))
        out_pool = ctx.enter_context(tc.tile_pool(name="out", bufs=4))
        psum_pool = ctx.enter_context(tc.tile_pool(name="psum", bufs=8, space="PSUM"))

        wm = wm_pool.tile([128, 384], BF16, name="wm")
        nc.vector.memset(wm[:], 0.0)

        # --- input DMAs, deadline-ordered. Each plane is split in half
        # across the two HWDGE rings (both rings fill one plane concurrently
        # so the next unit's slice arrives ~2x sooner); g pieces interleave on
        # the scalar ring by unit deadline.
        pl_t = pl_pool.tile([128, 10, JC, TOK], FP8, name="pl")
        g_t = g_pool.tile([128, NU, 2, NPASS, 128], FP8, name="g")
        nc.sync.dma_start(pl_t[:, 0, 0:2], pl_d[:, 0, 0:2])    # segc jc01
        nc.scalar.dma_start(g_t[:, 0:2], g_d[:, 0:2])          # lin tables
        nc.scalar.dma_start(pl_t[:, 0, 2:4], pl_d[:, 0, 2:4])  # segc jc23
        g_cuts = [2, 6, 10, 14, 20]
        gi = 0
        for k in range(7):                                     # onehot planes
            nc.sync.dma_start(pl_t[:, 1 + k, 0:2], pl_d[:, 1 + k, 0:2])
            nc.scalar.dma_start(pl_t[:, 1 + k, 2:4], pl_d[:, 1 + k, 2:4])
            if k % 2 == 1 and gi < 4:
                a, b = g_cuts[gi], g_cuts[gi + 1]
                nc.scalar.dma_start(g_t[:, a:b], g_d[:, a:b])
                gi += 1
        while gi < 4:
            a, b = g_cuts[gi], g_cuts[gi + 1]
            nc.scalar.dma_start(g_t[:, a:b], g_d[:, a:b])
            gi += 1
        sb_t = sb_pool.tile([128, 2 * NPASS], F32, name="sb")
        nc.gpsimd.dma_start(sb_t[:], sb_d[:])

        # ACT-built planes from segc: qp = (m-4)^2 (exact in e4m3), then
        # onehot m=8 = relu(1 - (m-8)^2) with a bf16 intermediate.
        sq8 = tmp_pool.tile([128, JC, TOK], BF16, name="sq8")
        cm4 = tmp_pool.tile([128, 1], F32, name="cm4")
        nc.vector.memset(cm4[:], -4.0)
        for jc in range(JC):
            nc.scalar.square(pl_t[:, 8, jc], pl_t[:, 0, jc])
        for jc in range(JC):
            nc.scalar.activation(
                sq8[:, jc], pl_t[:, 0, jc],
                mybir.ActivationFunctionType.Square, bias=cm4[:],
            )
        for jc in range(JC):
            nc.scalar.activation(
                pl_t[:, 9, jc], sq8[:, jc],
                mybir.ActivationFunctionType.Relu, bias=1.0, scale=-1.0,
            )

        def mm(ps, ob, u, tg):
            pk, q = _UNITS[u]
            nc.tensor.matmul(
                ps,
                g_t[:, u, :, ob, :],
                pl_t[:, pk, 2 * q:2 * q + 2, tg * TGW:(tg + 1) * TGW],
                start=(u == 0),
                stop=(u == NU - 1),
                perf_mode=mybir.MatmulPerfMode.DoubleRow,
            )

        def evac(ps, ob, tg):
            ot = out_pool.tile([128, TGW], BF16, name="ot")
            nc.vector.tensor_scalar(
                ot[:], ps[:], sb_t[:, ob:ob + 1],
                sb_t[:, NPASS + ob:NPASS + ob + 1],
                mybir.AluOpType.mult, mybir.AluOpType.add,
            )
            eng = nc.sync if ob % 2 == 0 else nc.scalar
            eng.dma_start(out_d[ob][:, tg * TGW:(tg + 1) * TGW], ot[:])

        # pair 0 (ob 0,1): unit-outer interleave — unit u's table/plane DMA
        # deadline is ~1.7us*u. ob0 leads ob1 by SKEW units so its psum
        # evacuations overlap ob1's stream.
        SKEW = 3
        pss = {
            ob: [psum_pool.tile([128, TGW], F32, name="ps") for _ in range(NTG)]
            for ob in (0, 1)
        }
        for _ in range(40):
            nc.tensor.matmul(
                pss[0][0][:, :256], wm[:, :128], wm[:, 128:384],
                start=True, stop=True, skip_group_check=True,
            )
        sched = [(0, u) for u in range(SKEW)]
        for u in range(NU):
            sched.append((1, u))
            if u + SKEW < NU:
                sched.append((0, u + SKEW))
        for ob, u in sched:
            for tg in range(NTG):
                mm(pss[ob][tg][:], ob, u, tg)
            if u == NU - 1:
                for tg in range(NTG):
                    evac(pss[ob][tg], ob, tg)

        # pair 1 (ob 2,3): all inputs resident — token-group-outer so each
        # single-bank psum completes early and output trickles out.
        for ob in (2, 3):
            for tg in range(NTG):
                ps = psum_pool.tile([128, TGW], F32, name="ps")
                for u in range(NU):
                    mm(ps[:], ob, u, tg)
                evac(ps, ob, tg)

    nc.compile()
    return nc


def _get_program():
    if "nc" not in _PROGRAM_CACHE:
        _PROGRAM_CACHE["nc"] = _build_program()
    return _PROGRAM_CACHE["nc"]


def _plane_dev(arr):
    """[T_all, IN] -> [128, JC, T_all] device layout (j = jc*128 + p)."""
    return np.ascontiguousarray(arr.T.reshape(JC, 128, -1).transpose(1, 0, 2))


def _pack_pair(tab_b):
    """e4m3 [OUT, IN] -> [128p, 2q, 2e, NPASS, 128col] stationary layout."""
    t = tab_b.reshape(NPASS, 128, JC, 128).transpose(3, 2, 0, 1)
    return np.ascontiguousarray(t.reshape(128, 2, 2, NPASS, 128))


def kernel(x: np.ndarray, coeffs: np.ndarray) -> np.ndarray:
    assert x.shape == (8, 2048, IN_F) and coeffs.shape == (OUT_F, IN_F, 12)
    t = np.linspace(0.0, 1.0, 10, dtype=np.float32)  # same knots as reference

    # Segment index via the same float32 comparisons the reference uses.
    xf = np.ascontiguousarray(x.reshape(-1, IN_F))          # [16384, 512]
    seg = np.zeros(xf.shape, dtype=np.int32)
    for m in range(1, 9):
        seg += (xf >= t[m]).astype(np.int32)

    # Table build (see module docstring): c0 = f(4); B2 quantized first
    # (absorbed); R8 next; B1 refit last, hi+lo.
    c = coeffs.astype(np.float64)
    F = np.stack(
        [c[:, :, m] + c[:, :, m + 1] + c[:, :, m + 2] for m in range(9)]
    ).reshape(9, -1)                                         # [9, OUT*IN]
    mc = np.arange(9.0) - 4.0
    qv = mc * mc
    D = F - F[4:5]
    Phi2 = np.stack([mc, qv], axis=1)                        # [9, 2]
    co = np.linalg.lstsq(Phi2, D, rcond=None)[0]
    r0 = (D - Phi2 @ co).reshape(9, OUT_F, IN_F)
    alpha = 240.0 / (1.02 * np.abs(r0).max(axis=(0, 2)))     # per-out-row
    a2 = alpha[:, None]
    a3 = alpha[None, :, None]

    def q8(v, a):
        return np.clip(v * a, -240.0, 240.0).astype(E4NP)

    B1, B2 = (co[k].reshape(OUT_F, IN_F) for k in range(2))
    B1b = q8(B1, a2)
    B2b = q8(B2, a2)
    B1q = B1b.astype(np.float64) / a2
    B2q = B2b.astype(np.float64) / a2
    res = (
        D.reshape(9, OUT_F, IN_F)
        - B1q[None] * mc[:, None, None]
        - B2q[None] * qv[:, None, None]
    )
    R8b = q8(res, a3)
    R8b[4] = 0

    g_dev = np.empty((128, NU, 2, NPASS, 128), dtype=E4NP)
    for u0, tab in ((0, B1b), (16, B2b), (18, R8b[8])):
        pk = _pack_pair(tab)
        for q in range(2):
            g_dev[:, u0 + q] = pk[:, q]
    for k, m in enumerate(OH_MS):
        pk = _pack_pair(R8b[m])
        for q in range(2):
            g_dev[:, 2 + 2 * k + q] = pk[:, q]
    g_dev = np.ascontiguousarray(g_dev)

    base = F[4].reshape(OUT_F, IN_F).sum(axis=1)             # exact fp32
    sb = np.empty((128, 2 * NPASS), dtype=np.float32)
    for ob in range(NPASS):
        sl = slice(ob * 128, (ob + 1) * 128)
        sb[:, ob] = (1.0 / alpha[sl]).astype(np.float32)
        sb[:, NPASS + ob] = base[sl]

    # Plane bytes via uint8 LUTs over seg (fast).
    planes = np.empty((128, 8, JC, seg.shape[0]), dtype=E4NP)
    lut_segc = mc.astype(E4NP).view(np.uint8)
    planes[:, 0] = _plane_dev(lut_segc[seg]).view(E4NP)
    for k, m in enumerate(OH_MS):
        lut = np.zeros(9, E4NP)
        lut[m] = 1.0
        planes[:, 1 + k] = _plane_dev(lut.view(np.uint8)[seg]).view(E4NP)

    in_maps = []
    for core in range(N_CORES):
        sl = slice(core * TOK, (core + 1) * TOK)
        in_maps.append(
            {
                "pl": np.ascontiguousarray(planes[:, :, :, sl]),
                "g": g_dev,
                "sb": sb,
            }
        )

    nc = _get_program()
    res_ = run_bass_kernel_spmd(nc, in_maps, core_ids=list(range(N_CORES)))
    out = np.stack(
        [
            res_.results[core]["out"].reshape(OUT_F, TOK).T.astype(np.float32)
            for core in range(N_CORES)
        ]
    )
    return np.ascontiguousarray(out)


# revision 14
# speedup vs baseline: 2.2151x; 1.0239x over previous
"""Trainium2 Bass kernel for nn_KANLayer (piecewise-constant KAN forward).

Math: reference computes out[t,i] = sum_j f[i,j,m(x_tj)] where m = segment(x)
in 0..8 and f[i,j,m] = c_m + c_{m+1} + c_{m+2} (9-valued selection -> exact
rank 8 + constant; the bf16 version needs K=4096 = 512 MMs at 216ns/core).

This kernel runs the whole contraction in fp8-e4m3 DoubleRow (2 fp8 weights
per PE cell -> K=256 per matmul at the same 216ns N=512 stream = 2x bf16
FLOPs), with the table split to keep e4m3 quantization noise in budget:

    out[t,i] = base_i + (1/a_i) * [ sum_{m!=4} R8[i,j,m] * onehot_m(t,j)  16 units
                                  + B1q[i,j] * (m_tj-4)                    2 units
                                  + B2q[i,j] * (m_tj-4)^2 ]                2 units

Table construction (host, f64) exploits quantization-error absorption:
c0 is pinned to f(4) (residual at m=4 is exactly zero -> the m=4 one-hot
plane and its 2 units are dropped); B1 and B2 are quantized FIRST
(single e4m3 pass each) so their quantization error is absorbed into the
later-quantized one-hot residual R8 — the absorption direction that keeps
total noise at 1.8e-2 with only one pass per affine plane. a_i is a per-output-row scale applied at
evacuation via an AP scalar. All plane values (0/1, m-4, (m-4)^2) are
fp8-exact; host ships raw e4m3 bytes. End-to-end noise on the reference
seed: 1.82e-2 (threshold 2e-2), verified by exact full-set host simulation.

Structure per core: 20 units x 4 out-blocks x 4 token-groups = 320 DR MMs at
216ns = 76us PE. PSUM is 8 single-bank [128,512] tiles so each token-group
slice's evacuation (DVE scale+bias -> bf16 -> DMA) never blocks the next
slice's accumulation. Pair 0 (out-blocks 0,1) runs unit-outer with ob0
skewed 3 units ahead (DMA-deadline-friendly while tables/planes stream in,
evacs hidden); pair 1 runs token-group-outer so the exposed tail is one
slice. The fp8 table pair is stationary, reused across 4 N=512 matmuls, so
the 256-col DR LDWEIGHTS (135ns) hides. The (m-4)^2 plane is squared from
the (m-4) plane on the otherwise-idle ACT engine; everything else is
host-shipped (device-side fp8 DVE/GpSimd builds measured 10-30x slower than
bf16 rates). Output leaves as [out_block, 128i, tok] bf16, upcast/transposed
on host. Sharding: data-parallel over tokens, 2048 per core; tables
replicated.
"""

from contextlib import ExitStack

import numpy as np

import concourse.bass as bass  # noqa: F401
import concourse.tile as tile
from concourse import bacc, mybir
from concourse.bass_utils import run_bass_kernel_spmd

N_CORES = 8
TOK = 2048          # tokens per core
IN_F = 512
OUT_F = 512
JC = IN_F // 128    # 4 j-chunks of 128
NPASS = OUT_F // 128  # 4 out-blocks
NTG = 4             # token groups (N=512 matmuls) per out-block
TGW = TOK // NTG
NU = 20             # DR units: 2 lin + 16 onehot (m!=4) + 2 quad
FP8 = mybir.dt.float8e4
BF16 = mybir.dt.bfloat16
F32 = mybir.dt.float32
E4NP = mybir.dt.np(FP8)  # ml_dtypes.float8_e4m3 (TRN: bias 7, max 240)

OH_MS = [0, 1, 2, 3, 5, 6]         # shipped one-hot planes (m=4 dropped)

# unit -> (plane index, jc-pair q). planes: 0=segc (m-4), 1..6=onehot for
# OH_MS, 7/8=onehot m=7/8 (DVE bf16 is_equal -> ACT fp8 convert), 9=qp
# ((m-4)^2, ACT square). lin first, shipped onehots ascending, device-built
# last in build-completion order (qp, m7, m8).
_UNITS = []
for q in range(2):
    _UNITS.append((0, q))
for k in range(6):
    for q in range(2):
        _UNITS.append((1 + k, q))
for pk in (9, 7, 8):
    for q in range(2):
        _UNITS.append((pk, q))
assert len(_UNITS) == NU

_PROGRAM_CACHE = {}


def _build_program():
    nc = bacc.Bacc("TRN2", target_bir_lowering=False, debug=False)

    pl_d = nc.dram_tensor("pl", [128, 7, JC, TOK], FP8, kind="ExternalInput").ap()
    g_d = nc.dram_tensor("g", [128, NU, 2, NPASS, 128], FP8, kind="ExternalInput").ap()
    sb_d = nc.dram_tensor("sb", [128, 2 * NPASS], F32, kind="ExternalInput").ap()
    out_d = nc.dram_tensor("out", [NPASS, 128, TOK], BF16, kind="ExternalOutput").ap()

    with tile.TileContext(nc) as tc, ExitStack() as ctx:
        wm_pool = ctx.enter_context(tc.tile_pool(name="wm", bufs=1))
        tmp_pool = ctx.enter_context(tc.tile_pool(name="tmp", bufs=2))
        pl_pool = ctx.enter_context(tc.tile_pool(name="pl", bufs=1))
        g_pool = ctx.enter_context(tc.tile_pool(name="g", bufs=1))
        sb_pool = ctx.enter_context(tc.tile_pool(name="sb", bufs=1))
        out_pool = ctx.enter_context(tc.tile_pool(name="out", bufs=4))
        psum_pool = ctx.enter_context(tc.tile_pool(name="psum", bufs=8, space="PSUM"))

        wm = wm_pool.tile([128, 384], BF16, name="wm")
        nc.vector.memset(wm[:], 0.0)

        # --- input DMAs, deadline-ordered across the two HWDGE rings (few,
        # large pieces — many small pieces thrash the 8 DMA sem lanes and
        # serialize the issue stream).
        pl_t = pl_pool.tile([128, 10, JC, TOK], FP8, name="pl")
        g_t = g_pool.tile([128, NU, 2, NPASS, 128], FP8, name="g")
        nc.sync.dma_start(pl_t[:, 0, 0:2], pl_d[:, 0, 0:2])    # segc jc01
        nc.scalar.dma_start(g_t[:, 0:2], g_d[:, 0:2])          # lin tables
        nc.scalar.dma_start(pl_t[:, 0, 2:4], pl_d[:, 0, 2:4])  # segc jc23
        nc.scalar.dma_start(g_t[:, 2:6], g_d[:, 2:6])
        nc.sync.dma_start(pl_t[:, 1], pl_d[:, 1])              # oh m0
        nc.scalar.dma_start(pl_t[:, 2], pl_d[:, 2])            # oh m1
        nc.scalar.dma_start(g_t[:, 6:10], g_d[:, 6:10])
        nc.sync.dma_start(pl_t[:, 3], pl_d[:, 3])              # oh m2
        nc.scalar.dma_start(pl_t[:, 4], pl_d[:, 4])            # oh m3
        nc.scalar.dma_start(g_t[:, 10:14], g_d[:, 10:14])
        nc.sync.dma_start(pl_t[:, 5], pl_d[:, 5])              # oh m5
        nc.scalar.dma_start(pl_t[:, 6], pl_d[:, 6])            # oh m6
        nc.scalar.dma_start(g_t[:, 14:20], g_d[:, 14:20])
        sb_t = sb_pool.tile([128, 2 * NPASS], F32, name="sb")
        nc.gpsimd.dma_start(sb_t[:], sb_d[:])

        # Device-built planes: qp = segc^2 on ACT; one-hot m7/m8 via DVE
        # bf16 is_equal (fast path) + ACT copy-convert to fp8.
        for jc in range(JC):
            nc.scalar.square(pl_t[:, 9, jc], pl_t[:, 0, jc])
        for slot, mval in ((7, 7), (8, 8)):
            tmp = tmp_pool.tile([128, JC, TOK], BF16, name="ohb")
            for jc in range(JC):
                nc.vector.tensor_scalar(
                    tmp[:, jc], pl_t[:, 0, jc],
                    float(mval - 4), None, mybir.AluOpType.is_equal,
                )
            for jc in range(JC):
                nc.scalar.activation(
                    pl_t[:, slot, jc], tmp[:, jc],
                    mybir.ActivationFunctionType.Copy,
                )

        def mm(ps, ob, u, tg):
            pk, q = _UNITS[u]
            nc.tensor.matmul(
                ps,
                g_t[:, u, :, ob, :],
                pl_t[:, pk, 2 * q:2 * q + 2, tg * TGW:(tg + 1) * TGW],
                start=(u == 0),
                stop=(u == NU - 1),
                perf_mode=mybir.MatmulPerfMode.DoubleRow,
            )

        def evac(ps, ob, tg, ot=None, dma=True):
            if ot is None:
                ot = out_pool.tile([128, TGW], BF16, name="ot")
                osl = ot[:]
            else:
                osl = ot[:, tg * TGW:(tg + 1) * TGW]
            nc.vector.tensor_scalar(
                osl, ps[:], sb_t[:, ob:ob + 1],
                sb_t[:, NPASS + ob:NPASS + ob + 1],
                mybir.AluOpType.mult, mybir.AluOpType.add,
            )
            eng = nc.sync if ob % 2 == 0 else nc.scalar
            if dma:
                eng.dma_start(out_d[ob][:, tg * TGW:(tg + 1) * TGW], osl)

        # pair 0 (ob 0,1): unit-outer interleave — unit u's table/plane DMA
        # deadline is ~1.7us*u. ob0 leads ob1 by SKEW units so its psum
        # evacuations overlap ob1's stream.
        SKEW = 3
        pss = {
            ob: [psum_pool.tile([128, TGW], F32, name="ps") for _ in range(NTG)]
            for ob in (0, 1)
        }
        for _ in range(40):
            nc.tensor.matmul(
                pss[0][0][:, :256], wm[:, :128], wm[:, 128:384],
                start=True, stop=True, skip_group_check=True,
            )
        sched = [(0, u) for u in range(SKEW)]
        for u in range(NU):
            sched.append((1, u))
            if u + SKEW < NU:
                sched.append((0, u + SKEW))
        for ob, u in sched:
            for tg in range(NTG):
                mm(pss[ob][tg][:], ob, u, tg)
            if u == NU - 1:
                otb = out_pool.tile([128, TOK], BF16, name="otb")
                for tg in range(NTG):
                    evac(pss[ob][tg], ob, tg, ot=otb, dma=False)
                eng = nc.sync if ob % 2 == 0 else nc.scalar
                eng.dma_start(out_d[ob], otb[:])

        # pair 1 (ob 2,3): all inputs resident — token-group-outer so each
        # single-bank psum completes early and output trickles out.
        for ob in (2, 3):
            for tg in range(NTG):
                ps = psum_pool.tile([128, TGW], F32, name="ps")
                for u in range(NU):
                    mm(ps[:], ob, u, tg)
                evac(ps, ob, tg)

    nc.compile()
    return nc


def _get_program():
    if "nc" not in _PROGRAM_CACHE:
        _PROGRAM_CACHE["nc"] = _build_program()
    return _PROGRAM_CACHE["nc"]


def _plane_dev(arr):
    """[T_all, IN] -> [128, JC, T_all] device layout (j = jc*128 + p)."""
    return np.ascontiguousarray(arr.T.reshape(JC, 128, -1).transpose(1, 0, 2))


def _pack_pair(tab_b):
    """e4m3 [OUT, IN] -> [128p, 2q, 2e, NPASS, 128col] stationary layout."""
    t = tab_b.reshape(NPASS, 128, JC, 128).transpose(3, 2, 0, 1)
    return np.ascontiguousarray(t.reshape(128, 2, 2, NPASS, 128))


def kernel(x: np.ndarray, coeffs: np.ndarray) -> np.ndarray:
    assert x.shape == (8, 2048, IN_F) and coeffs.shape == (OUT_F, IN_F, 12)
    t = np.linspace(0.0, 1.0, 10, dtype=np.float32)  # same knots as reference

    # Segment index via the same float32 comparisons the reference uses.
    xf = np.ascontiguousarray(x.reshape(-1, IN_F))          # [16384, 512]
    seg = np.zeros(xf.shape, dtype=np.int32)
    for m in range(1, 9):
        seg += (xf >= t[m]).astype(np.int32)

    # Table build (see module docstring): c0 = f(4); B2 quantized first
    # (absorbed); R8 next; B1 refit last, hi+lo.
    c = coeffs.astype(np.float64)
    F = np.stack(
        [c[:, :, m] + c[:, :, m + 1] + c[:, :, m + 2] for m in range(9)]
    ).reshape(9, -1)                                         # [9, OUT*IN]
    mc = np.arange(9.0) - 4.0
    qv = mc * mc
    D = F - F[4:5]
    Phi2 = np.stack([mc, qv], axis=1)                        # [9, 2]
    co = np.linalg.lstsq(Phi2, D, rcond=None)[0]
    r0 = (D - Phi2 @ co).reshape(9, OUT_F, IN_F)
    alpha = 240.0 / (1.02 * np.abs(r0).max(axis=(0, 2)))     # per-out-row
    a2 = alpha[:, None]
    a3 = alpha[None, :, None]

    def q8(v, a):
        return np.clip(v * a, -240.0, 240.0).astype(E4NP)

    B1, B2 = (co[k].reshape(OUT_F, IN_F) for k in range(2))
    B1b = q8(B1, a2)
    B2b = q8(B2, a2)
    B1q = B1b.astype(np.float64) / a2
    B2q = B2b.astype(np.float64) / a2
    res = (
        D.reshape(9, OUT_F, IN_F)
        - B1q[None] * mc[:, None, None]
        - B2q[None] * qv[:, None, None]
    )
    R8b = q8(res, a3)
    R8b[4] = 0

    g_dev = np.empty((128, NU, 2, NPASS, 128), dtype=E4NP)
    for u0, tab in ((0, B1b), (14, B2b), (16, R8b[7]), (18, R8b[8])):
        pk = _pack_pair(tab)
        for q in range(2):
            g_dev[:, u0 + q] = pk[:, q]
    for k, m in enumerate(OH_MS):
        pk = _pack_pair(R8b[m])
        for q in range(2):
            g_dev[:, 2 + 2 * k + q] = pk[:, q]
    g_dev = np.ascontiguousarray(g_dev)

    base = F[4].reshape(OUT_F, IN_F).sum(axis=1)             # exact fp32
    sb = np.empty((128, 2 * NPASS), dtype=np.float32)
    for ob in range(NPASS):
        sl = slice(ob * 128, (ob + 1) * 128)
        sb[:, ob] = (1.0 / alpha[sl]).astype(np.float32)
        sb[:, NPASS + ob] = base[sl]

    # Plane bytes via uint8 LUTs over seg (fast).
    planes = np.empty((128, 7, JC, seg.shape[0]), dtype=E4NP)
    lut_segc = mc.astype(E4NP).view(np.uint8)
    planes[:, 0] = _plane_dev(lut_segc[seg]).view(E4NP)
    for k, m in enumerate(OH_MS):
        lut = np.zeros(9, E4NP)
        lut[m] = 1.0
        planes[:, 1 + k] = _plane_dev(lut.view(np.uint8)[seg]).view(E4NP)

    in_maps = []
    for core in range(N_CORES):
        sl = slice(core * TOK, (core + 1) * TOK)
        in_maps.append(
            {
                "pl": np.ascontiguousarray(planes[:, :, :, sl]),
                "g": g_dev,
                "sb": sb,
            }
        )

    nc = _get_program()
    res_ = run_bass_kernel_spmd(nc, in_maps, core_ids=list(range(N_CORES)))
    out = np.stack(
        [
            res_.results[core]["out"].reshape(OUT_F, TOK).T.astype(np.float32)
            for core in range(N_CORES)
        ]
    )
    return np.ascontiguousarray(out)


# revision 15
# speedup vs baseline: 2.2374x; 1.0101x over previous
"""Trainium2 Bass kernel for nn_KANLayer (piecewise-constant KAN forward).

Math: reference computes out[t,i] = sum_j f[i,j,m(x_tj)] where m = segment(x)
in 0..8 and f[i,j,m] = c_m + c_{m+1} + c_{m+2} (9-valued selection -> exact
rank 8 + constant; the bf16 version needs K=4096 = 512 MMs at 216ns/core).

This kernel runs the whole contraction in fp8-e4m3 DoubleRow (2 fp8 weights
per PE cell -> K=256 per matmul at the same 216ns N=512 stream = 2x bf16
FLOPs), with the table split to keep e4m3 quantization noise in budget:

    out[t,i] = base_i + (1/a_i) * [ sum_{m!=4} R8[i,j,m] * onehot_m(t,j)  16 units
                                  + B1q[i,j] * (m_tj-4)                    2 units
                                  + B2q[i,j] * (m_tj-4)^2 ]                2 units

Table construction (host, f64) exploits quantization-error absorption:
c0 is pinned to f(4) (residual at m=4 is exactly zero -> the m=4 one-hot
plane and its 2 units are dropped); B1 and B2 are quantized FIRST
(single e4m3 pass each) so their quantization error is absorbed into the
later-quantized one-hot residual R8 — the absorption direction that keeps
total noise at 1.8e-2 with only one pass per affine plane. a_i is a per-output-row scale applied at
evacuation via an AP scalar. All plane values (0/1, m-4, (m-4)^2) are
fp8-exact; host ships raw e4m3 bytes. End-to-end noise on the reference
seed: 1.82e-2 (threshold 2e-2), verified by exact full-set host simulation.

Structure per core: 20 units x 4 out-blocks x 4 token-groups = 320 DR MMs at
216ns = 76us PE. PSUM is 8 single-bank [128,512] tiles so each token-group
slice's evacuation (DVE scale+bias -> bf16 -> DMA) never blocks the next
slice's accumulation. Pair 0 (out-blocks 0,1) runs unit-outer with ob0
skewed 3 units ahead (DMA-deadline-friendly while tables/planes stream in,
evacs hidden); pair 1 runs token-group-outer so the exposed tail is one
slice. The fp8 table pair is stationary, reused across 4 N=512 matmuls, so
the 256-col DR LDWEIGHTS (135ns) hides. The (m-4)^2 plane is squared from
the (m-4) plane on the otherwise-idle ACT engine; everything else is
host-shipped (device-side fp8 DVE/GpSimd builds measured 10-30x slower than
bf16 rates). Output leaves as [out_block, 128i, tok] bf16, upcast/transposed
on host. Sharding: data-parallel over tokens, 2048 per core; tables
replicated.
"""

from contextlib import ExitStack

import numpy as np

import concourse.bass as bass  # noqa: F401
import concourse.tile as tile
from concourse import bacc, mybir
from concourse.bass_utils import run_bass_kernel_spmd

N_CORES = 8
TOK = 2048          # tokens per core
IN_F = 512
OUT_F = 512
JC = IN_F // 128    # 4 j-chunks of 128
NPASS = OUT_F // 128  # 4 out-blocks
NTG = 4             # token groups (N=512 matmuls) per out-block
TGW = TOK // NTG
NU = 20             # DR units: 2 lin + 16 onehot (m!=4) + 2 quad
FP8 = mybir.dt.float8e4
BF16 = mybir.dt.bfloat16
F32 = mybir.dt.float32
E4NP = mybir.dt.np(FP8)  # ml_dtypes.float8_e4m3 (TRN: bias 7, max 240)

OH_MS = [0, 1, 2, 3, 5, 6]         # shipped one-hot planes (m=4 dropped)

# unit -> (plane index, jc-pair q). planes: 0=segc (m-4), 1..6=onehot for
# OH_MS, 7/8=onehot m=7/8 (DVE bf16 is_equal -> ACT fp8 convert), 9=qp
# ((m-4)^2, ACT square). lin first, shipped onehots ascending, device-built
# last in build-completion order (qp, m7, m8).
_UNITS = []
for q in range(2):
    _UNITS.append((0, q))
for k in range(6):
    for q in range(2):
        _UNITS.append((1 + k, q))
for pk in (9, 7, 8):
    for q in range(2):
        _UNITS.append((pk, q))
assert len(_UNITS) == NU

_PROGRAM_CACHE = {}


def _build_program():
    nc = bacc.Bacc("TRN2", target_bir_lowering=False, debug=False)

    pl_d = nc.dram_tensor("pl", [128, 7, JC, TOK], FP8, kind="ExternalInput").ap()
    g_d = nc.dram_tensor("g", [128, NU, 2, NPASS, 128], FP8, kind="ExternalInput").ap()
    sb_d = nc.dram_tensor("sb", [128, 2 * NPASS], F32, kind="ExternalInput").ap()
    out_d = nc.dram_tensor("out", [NPASS, 128, TOK], BF16, kind="ExternalOutput").ap()

    with tile.TileContext(nc) as tc, ExitStack() as ctx:
        wm_pool = ctx.enter_context(tc.tile_pool(name="wm", bufs=1))
        tmp_pool = ctx.enter_context(tc.tile_pool(name="tmp", bufs=2))
        pl_pool = ctx.enter_context(tc.tile_pool(name="pl", bufs=1))
        g_pool = ctx.enter_context(tc.tile_pool(name="g", bufs=1))
        sb_pool = ctx.enter_context(tc.tile_pool(name="sb", bufs=1))
        out_pool = ctx.enter_context(tc.tile_pool(name="out", bufs=4))
        psum_pool = ctx.enter_context(tc.tile_pool(name="psum", bufs=8, space="PSUM"))

        wm = wm_pool.tile([128, 384], BF16, name="wm")
        nc.vector.memset(wm[:], 0.0)

        # --- input DMAs, deadline-ordered across the two HWDGE rings (few,
        # large pieces — many small pieces thrash the 8 DMA sem lanes and
        # serialize the issue stream).
        pl_t = pl_pool.tile([128, 10, JC, TOK], FP8, name="pl")
        g_t = g_pool.tile([128, NU, 2, NPASS, 128], FP8, name="g")
        nc.sync.dma_start(pl_t[:, 0, 0:2], pl_d[:, 0, 0:2])    # segc jc01
        nc.scalar.dma_start(g_t[:, 0:2], g_d[:, 0:2])          # lin tables
        nc.scalar.dma_start(pl_t[:, 0, 2:4], pl_d[:, 0, 2:4])  # segc jc23
        nc.scalar.dma_start(g_t[:, 2:6], g_d[:, 2:6])
        nc.sync.dma_start(pl_t[:, 1], pl_d[:, 1])              # oh m0
        nc.scalar.dma_start(pl_t[:, 2], pl_d[:, 2])            # oh m1
        nc.scalar.dma_start(g_t[:, 6:10], g_d[:, 6:10])
        nc.sync.dma_start(pl_t[:, 3], pl_d[:, 3])              # oh m2
        nc.scalar.dma_start(pl_t[:, 4], pl_d[:, 4])            # oh m3
        nc.scalar.dma_start(g_t[:, 10:14], g_d[:, 10:14])
        nc.sync.dma_start(pl_t[:, 5], pl_d[:, 5])              # oh m5
        nc.scalar.dma_start(pl_t[:, 6], pl_d[:, 6])            # oh m6
        nc.scalar.dma_start(g_t[:, 14:20], g_d[:, 14:20])
        sb_t = sb_pool.tile([128, 2 * NPASS], F32, name="sb")
        nc.gpsimd.dma_start(sb_t[:], sb_d[:])

        # Device-built planes: qp = segc^2 on ACT; one-hot m7/m8 via DVE
        # bf16 is_equal (fast path) + ACT copy-convert to fp8.
        for jc in range(JC):
            nc.scalar.square(pl_t[:, 9, jc], pl_t[:, 0, jc])
        for slot, mval in ((7, 7), (8, 8)):
            tmp = tmp_pool.tile([128, JC, TOK], BF16, name="ohb")
            for jc in range(JC):
                nc.vector.tensor_scalar(
                    tmp[:, jc], pl_t[:, 0, jc],
                    float(mval - 4), None, mybir.AluOpType.is_equal,
                )
            for jc in range(JC):
                nc.scalar.activation(
                    pl_t[:, slot, jc], tmp[:, jc],
                    mybir.ActivationFunctionType.Copy,
                )

        def mm(ps, ob, u, tg):
            pk, q = _UNITS[u]
            nc.tensor.matmul(
                ps,
                g_t[:, u, :, ob, :],
                pl_t[:, pk, 2 * q:2 * q + 2, tg * TGW:(tg + 1) * TGW],
                start=(u == 0),
                stop=(u == NU - 1),
                perf_mode=mybir.MatmulPerfMode.DoubleRow,
            )

        def evac(ps, ob, tg, ot=None, dma=True):
            if ot is None:
                ot = out_pool.tile([128, TGW], BF16, name="ot")
                osl = ot[:]
            else:
                osl = ot[:, tg * TGW:(tg + 1) * TGW]
            nc.vector.tensor_scalar(
                osl, ps[:], sb_t[:, ob:ob + 1],
                sb_t[:, NPASS + ob:NPASS + ob + 1],
                mybir.AluOpType.mult, mybir.AluOpType.add,
            )
            eng = nc.sync if ob % 2 == 0 else nc.scalar
            if dma:
                eng.dma_start(out_d[ob][:, tg * TGW:(tg + 1) * TGW], osl)

        # pair 0 (ob 0,1): unit-outer interleave — unit u's table/plane DMA
        # deadline is ~1.7us*u. ob0 leads ob1 by SKEW units so its psum
        # evacuations overlap ob1's stream.
        SKEW = 3
        pss = {
            ob: [psum_pool.tile([128, TGW], F32, name="ps") for _ in range(NTG)]
            for ob in (0, 1)
        }
        for _ in range(40):
            nc.tensor.matmul(
                pss[0][0][:, :256], wm[:, :128], wm[:, 128:384],
                start=True, stop=True, skip_group_check=True,
            )
        sched = [(0, u) for u in range(SKEW)]
        for u in range(NU):
            sched.append((1, u))
            if u + SKEW < NU:
                sched.append((0, u + SKEW))
        for si, (ob, u) in enumerate(sched):
            if 1 <= si <= 3:
                # warmup bursts between the first units: the early stream is
                # DMA-ramp-gated; keep the PE busy so HAM stays at 8/8.
                # Target ob1's last tile — its real (start=True) group opens
                # later, at sched entry (1, 0).
                for _ in range(16):
                    nc.tensor.matmul(
                        pss[1][3][:, :256], wm[:, :128], wm[:, 128:384],
                        start=True, stop=True, skip_group_check=True,
                    )
            for tg in range(NTG):
                mm(pss[ob][tg][:], ob, u, tg)
            if u == NU - 1:
                otb = out_pool.tile([128, TOK], BF16, name="otb")
                for tg in range(NTG):
                    evac(pss[ob][tg], ob, tg, ot=otb, dma=False)
                eng = nc.sync if ob % 2 == 0 else nc.scalar
                eng.dma_start(out_d[ob], otb[:])

        # pair 1 (ob 2,3): all inputs resident — token-group-outer so each
        # single-bank psum completes early and output trickles out.
        for ob in (2, 3):
            for tg in range(NTG):
                ps = psum_pool.tile([128, TGW], F32, name="ps")
                for u in range(NU):
                    mm(ps[:], ob, u, tg)
                evac(ps, ob, tg)

    nc.compile()
    return nc


def _get_program():
    if "nc" not in _PROGRAM_CACHE:
        _PROGRAM_CACHE["nc"] = _build_program()
    return _PROGRAM_CACHE["nc"]


def _plane_dev(arr):
    """[T_all, IN] -> [128, JC, T_all] device layout (j = jc*128 + p)."""
    return np.ascontiguousarray(arr.T.reshape(JC, 128, -1).transpose(1, 0, 2))


def _pack_pair(tab_b):
    """e4m3 [OUT, IN] -> [128p, 2q, 2e, NPASS, 128col] stationary layout."""
    t = tab_b.reshape(NPASS, 128, JC, 128).transpose(3, 2, 0, 1)
    return np.ascontiguousarray(t.reshape(128, 2, 2, NPASS, 128))


def kernel(x: np.ndarray, coeffs: np.ndarray) -> np.ndarray:
    assert x.shape == (8, 2048, IN_F) and coeffs.shape == (OUT_F, IN_F, 12)
    t = np.linspace(0.0, 1.0, 10, dtype=np.float32)  # same knots as reference

    # Segment index via the same float32 comparisons the reference uses.
    xf = np.ascontiguousarray(x.reshape(-1, IN_F))          # [16384, 512]
    seg = np.zeros(xf.shape, dtype=np.int32)
    for m in range(1, 9):
        seg += (xf >= t[m]).astype(np.int32)

    # Table build (see module docstring): c0 = f(4); B2 quantized first
    # (absorbed); R8 next; B1 refit last, hi+lo.
    c = coeffs.astype(np.float64)
    F = np.stack(
        [c[:, :, m] + c[:, :, m + 1] + c[:, :, m + 2] for m in range(9)]
    ).reshape(9, -1)                                         # [9, OUT*IN]
    mc = np.arange(9.0) - 4.0
    qv = mc * mc
    D = F - F[4:5]
    Phi2 = np.stack([mc, qv], axis=1)                        # [9, 2]
    co = np.linalg.lstsq(Phi2, D, rcond=None)[0]
    r0 = (D - Phi2 @ co).reshape(9, OUT_F, IN_F)
    alpha = 240.0 / (1.02 * np.abs(r0).max(axis=(0, 2)))     # per-out-row
    a2 = alpha[:, None]
    a3 = alpha[None, :, None]

    def q8(v, a):
        return np.clip(v * a, -240.0, 240.0).astype(E4NP)

    B1, B2 = (co[k].reshape(OUT_F, IN_F) for k in range(2))
    B1b = q8(B1, a2)
    B2b = q8(B2, a2)
    B1q = B1b.astype(np.float64) / a2
    B2q = B2b.astype(np.float64) / a2
    res = (
        D.reshape(9, OUT_F, IN_F)
        - B1q[None] * mc[:, None, None]
        - B2q[None] * qv[:, None, None]
    )
    R8b = q8(res, a3)
    R8b[4] = 0

    g_dev = np.empty((128, NU, 2, NPASS, 128), dtype=E4NP)
    for u0, tab in ((0, B1b), (14, B2b), (16, R8b[7]), (18, R8b[8])):
        pk = _pack_pair(tab)
        for q in range(2):
            g_dev[:, u0 + q] = pk[:, q]
    for k, m in enumerate(OH_MS):
        pk = _pack_pair(R8b[m])
        for q in range(2):
            g_dev[:, 2 + 2 * k + q] = pk[:, q]
    g_dev = np.ascontiguousarray(g_dev)

    base = F[4].reshape(OUT_F, IN_F).sum(axis=1)             # exact fp32
    sb = np.empty((128, 2 * NPASS), dtype=np.float32)
    for ob in range(NPASS):
        sl = slice(ob * 128, (ob + 1) * 128)
        sb[:, ob] = (1.0 / alpha[sl]).astype(np.float32)
        sb[:, NPASS + ob] = base[sl]

    # Plane bytes via uint8 LUTs over seg (fast).
    planes = np.empty((128, 7, JC, seg.shape[0]), dtype=E4NP)
    lut_segc = mc.astype(E4NP).view(np.uint8)
    planes[:, 0] = _plane_dev(lut_segc[seg]).view(E4NP)
    for k, m in enumerate(OH_MS):
        lut = np.zeros(9, E4NP)
        lut[m] = 1.0
        planes[:, 1 + k] = _plane_dev(lut.view(np.uint8)[seg]).view(E4NP)

    in_maps = []
    for core in range(N_CORES):
        sl = slice(core * TOK, (core + 1) * TOK)
        in_maps.append(
            {
                "pl": np.ascontiguousarray(planes[:, :, :, sl]),
                "g": g_dev,
                "sb": sb,
            }
        )

    nc = _get_program()
    res_ = run_bass_kernel_spmd(nc, in_maps, core_ids=list(range(N_CORES)))
    out = np.stack(
        [
            res_.results[core]["out"].reshape(OUT_F, TOK).T.astype(np.float32)
            for core in range(N_CORES)
        ]
    )
    return np.ascontiguousarray(out)
